# revision 1
# baseline (speedup 1.0000x reference)
"""Multi-head rotary attention block on 8 Trainium2 NeuronCores.

Sharding: tensor-parallel over heads (16 heads / 8 cores = 2 heads per core,
both batches on every core); one 8-way AllToAll redistributes the attention
output from head-sharded to sequence-sharded for the output projection, so
each core finishes layernorm on its own [512, 1024] output slice.

Per-core dataflow (feature-major "T" layouts are [channels, seq]):
  qT/kT = w_q^T x^T (+bias via K=1 matmul) with RoPE applied as
          raw*cosM + (SpermT^T raw)*sinM (rotation permutation as a matmul)
  vT    = w_v^T x^T, PE-transposed to natural v [seq, 128]
  per head: sT[j,i] = kT^T qT (transposed scores), pT = exp(sT/32) on ScalarE
  (no max subtraction: scores are O(0.5) under this operator's input law),
  softmax denominators D via ones-matmul over pT, 1/D = exp(-ln(D)) on ScalarE,
  xvT = (v^T pT) * (1/D); AllToAll; z = xv_gathered + x residual;
  y = z^T w_out + b_out; layernorm via bn_stats/bn_aggr + exp(-0.5 ln(var+eps)).

All matmuls run as float32r (full-rate fp32 PE mode; 4-byte data, producers
write into tiles declared float32r as the BIR verifier requires).
"""
import numpy as np

import concourse.bass as bass
import concourse.bacc as bacc
import concourse.tile as tile
import concourse.mybir as mybir
from concourse import bass_utils

F32 = mybir.dt.float32
F32R = mybir.dt.float32r
AF = mybir.ActivationFunctionType
ALU = mybir.AluOpType

NCORES = 8
B, S, D = 2, 2048, 1024
HEADS, HD = 16, 64
SCALE = 1.0 / float(np.sqrt(D))  # reference scales by full D, not head_dim
IT = 512          # i-tile width for attention
N_IT = S // IT    # 4
JC = 128          # j-chunk
N_JC = S // JC    # 16
N_EC = D // 128   # 8 e-chunks


def _rope_consts():
    rot = HD // 2
    inv_freq = 1.0 / (10000.0 ** (np.arange(0, rot, 2, dtype=np.float64) / rot))
    ang = np.arange(S, dtype=np.float64)[:, None] * inv_freq[None, :]
    ang = np.repeat(ang, 2, axis=-1)  # [S, 32]
    cos, sin = np.cos(ang), np.sin(ang)
    cosM = np.ones((128, S), dtype=np.float32)
    sinM = np.zeros((128, S), dtype=np.float32)
    for base in (0, 64):
        cosM[base : base + 32, :] = cos.T.astype(np.float32)
        sinM[base : base + 32, :] = sin.T.astype(np.float32)
    Sp = np.zeros((128, 128), dtype=np.float32)
    for base in (0, 64):
        for m in range(32):
            r0 = base + m
            if m % 2 == 0:
                Sp[r0, r0 + 1] = -1.0
            else:
                Sp[r0, r0 - 1] = 1.0
    SpermT = np.ascontiguousarray(Sp.T)
    return cosM, sinM, SpermT


def _build(sim=False):
    nc = bacc.Bacc("TRN2", target_bir_lowering=False, debug=False, num_devices=NCORES)

    xT_d = [nc.dram_tensor(f"xT{b}", [D, S], F32R, kind="ExternalInput") for b in range(B)]
    wq_d = nc.dram_tensor("wq", [D, 128], F32R, kind="ExternalInput")
    wk_d = nc.dram_tensor("wk", [D, 128], F32R, kind="ExternalInput")
    wv_d = nc.dram_tensor("wv", [D, 128], F32R, kind="ExternalInput")
    bq_d = nc.dram_tensor("bq", [1, 128], F32R, kind="ExternalInput")
    bk_d = nc.dram_tensor("bk", [1, 128], F32R, kind="ExternalInput")
    bv_d = nc.dram_tensor("bv", [1, 128], F32R, kind="ExternalInput")
    wout_d = nc.dram_tensor("wout", [D, D], F32R, kind="ExternalInput")
    bout_d = nc.dram_tensor("bout", [1, D], F32R, kind="ExternalInput")
    gamma_d = nc.dram_tensor("gamma", [1, D], F32R, kind="ExternalInput")
    beta_d = nc.dram_tensor("beta", [1, D], F32R, kind="ExternalInput")
    cosM_d = nc.dram_tensor("cosM", [128, S], F32, kind="ExternalInput")
    sinM_d = nc.dram_tensor("sinM", [128, S], F32, kind="ExternalInput")
    spt_d = nc.dram_tensor("SpermT", [128, 128], F32R, kind="ExternalInput")
    ident_d = nc.dram_tensor("ident", [128, 128], F32, kind="ExternalInput")
    xres_d = nc.dram_tensor("xres", [D, 512], F32, kind="ExternalInput")
    y_d = nc.dram_tensor("y_out", [512, D], F32, kind="ExternalOutput")

    with tile.TileContext(nc) as tc:
        with (
            tc.tile_pool(name="persist", bufs=1) as pp,
            tc.tile_pool(name="dram", bufs=1, space="DRAM") as dram,
        ):
            cosM = pp.tile([128, S], F32, name="cosM_sb")
            sinM = pp.tile([128, S], F32, name="sinM_sb")
            spt = pp.tile([128, 128], F32R, name="spt_sb")
            ident = pp.tile([128, 128], F32, name="ident_sb")
            nc.sync.dma_start(cosM[:], cosM_d.ap())
            nc.sync.dma_start(sinM[:], sinM_d.ap())
            nc.sync.dma_start(spt[:], spt_d.ap())
            nc.sync.dma_start(ident[:], ident_d.ap())

            wq = pp.tile([128, N_EC, 128], F32R, name="wq_sb")  # [p, ec, m]
            wk = pp.tile([128, N_EC, 128], F32R, name="wk_sb")
            wv = pp.tile([128, N_EC, 128], F32R, name="wv_sb")
            for w_sb, w_dd in ((wq, wq_d), (wk, wk_d), (wv, wv_d)):
                nc.sync.dma_start(
                    w_sb[:], w_dd.ap().rearrange("(c p) m -> p c m", p=128))
            bq = pp.tile([1, 128], F32R, name="bq_sb")
            bk = pp.tile([1, 128], F32R, name="bk_sb")
            bv = pp.tile([1, 128], F32R, name="bv_sb")
            nc.sync.dma_start(bq[:], bq_d.ap())
            nc.sync.dma_start(bk[:], bk_d.ap())
            nc.sync.dma_start(bv[:], bv_d.ap())
            ones_row = pp.tile([1, IT], F32R, name="ones_row")
            nc.vector.memset(ones_row[:].bitcast(F32), 1.0)
            ones128 = pp.tile([128, 128], F32R, name="ones128")
            nc.vector.memset(ones128[:].bitcast(F32), 1.0)
            ones_col = pp.tile([1, 128], F32R, name="ones_col")
            nc.vector.memset(ones_col[:].bitcast(F32), 1.0)

            xvT = [pp.tile([128, S], F32, name=f"xvT_{b}") for b in range(B)]

            with (
                tc.tile_pool(name="psp", bufs=1, space="PSUM") as psp,
                tc.tile_pool(name="psa", bufs=1, space="PSUM") as psa,
                tc.tile_pool(name="ptmp", bufs=3) as ptmp,
                tc.tile_pool(name="ptp", bufs=8) as ptp,
                tc.tile_pool(name="rp", bufs=3) as rp,
            ):
             for b in range(B):
              with tc.tile_pool(name=f"qkv{b}", bufs=1) as qkvp:
                qTb = qkvp.tile([128, S], F32R, name=f"qT_{b}")
                kTb = qkvp.tile([128, S], F32R, name=f"kT_{b}")
                vnatb = [qkvp.tile([128, 130], F32R, name=f"v_{b}_{j}")
                         for j in range(N_JC)]
                for j in range(N_JC):
                    nc.vector.memset(vnatb[j][:, 64:65].bitcast(F32), 1.0)
                    nc.vector.memset(vnatb[j][:, 129:130].bitcast(F32), 1.0)
                # ---------- projections + rope for batch b ----------
                with (
                    tc.tile_pool(name=f"xt{b}", bufs=1) as xtp,
                ):
                    xt = xtp.tile([128, N_EC, S], F32R, name=f"xt_{b}")
                    xt_src = xT_d[b].ap().rearrange("(c p) s -> p c s", p=128)
                    for e in range(N_EC):
                        nc.sync.dma_start(xt[:, e, :], xt_src[:, e, :])

                    for w_sb, b_sb, dst in ((wq, bq, qTb), (wk, bk, kTb)):
                        for it in range(N_IT):
                            isl = slice(IT * it, IT * it + IT)
                            praw = psp.tile([128, IT], F32, tag="pqk")
                            for e in range(N_EC):
                                nc.tensor.matmul(
                                    praw[:], w_sb[:, e, :], xt[:, e, isl],
                                    start=(e == 0), stop=False)
                            nc.tensor.matmul(praw[:], b_sb[:], ones_row[:],
                                             start=False, stop=True)
                            raw = ptmp.tile([128, IT], F32R, tag="raw")
                            nc.vector.tensor_copy(raw[:], praw[:])
                            prot = psp.tile([128, IT], F32, tag="aux")
                            nc.tensor.matmul(prot[:], spt[:], raw[:],
                                             start=True, stop=True)
                            t1 = ptmp.tile([128, IT], F32, tag="t1")
                            nc.vector.tensor_tensor(t1[:], prot[:], sinM[:, isl], ALU.mult)
                            t2 = ptmp.tile([128, IT], F32, tag="t2")
                            nc.vector.tensor_tensor(
                                t2[:], raw[:].bitcast(F32), cosM[:, isl], ALU.mult)
                            nc.vector.tensor_tensor(dst[:, isl], t1[:], t2[:], ALU.add)

                    for it in range(N_IT):
                        isl = slice(IT * it, IT * it + IT)
                        pvt = psp.tile([128, IT], F32, tag="pqk")
                        for e in range(N_EC):
                            nc.tensor.matmul(pvt[:], wv[:, e, :], xt[:, e, isl],
                                             start=(e == 0), stop=False)
                        nc.tensor.matmul(pvt[:], bv[:], ones_row[:],
                                         start=False, stop=True)
                        vt_sb = ptmp.tile([128, IT], F32, tag="vt")
                        nc.vector.tensor_copy(vt_sb[:], pvt[:])
                        for jj in range(IT // 128):
                            jcc = it * (IT // 128) + jj
                            ptr_t = psp.tile([128, IT], F32, tag="aux", name="ptr_t")
                            ptr = ptr_t[:, 0:128]
                            nc.tensor.transpose(
                                ptr[:], vt_sb[:, 128 * jj : 128 * jj + 128], ident[:])
                            nc.vector.tensor_copy(vnatb[jcc][:, 0:64], ptr[:, 0:64])
                            nc.vector.tensor_copy(vnatb[jcc][:, 65:129], ptr[:, 64:128])

                # ---------- attention for batch b ----------
                if True:
                    for it in range(N_IT):
                        isl = slice(IT * it, IT * it + IT)
                        pxv = [psa.tile([128, IT], F32, tag="xv", bufs=2, name=f"pxv{_h}") for _h in range(2)]
                        for jc in range(N_JC):
                            jsl = slice(JC * jc, JC * jc + JC)
                            psc = [psa.tile([128, IT], F32, tag="sc", bufs=3, name=f"psc{_h}") for _h in range(2)]
                            pt = [ptp.tile([128, IT], F32R, tag="pt", name=f"pt{_h}") for _h in range(2)]
                            for hh in range(2):
                                hsl = slice(64 * hh, 64 * hh + 64)
                                nc.tensor.matmul(psc[hh][:], kTb[hsl, jsl],
                                                 qTb[hsl, isl], start=True, stop=True)
                                nc.scalar.activation(pt[hh][:], psc[hh][:], AF.Exp,
                                                     scale=SCALE)
                            first, last = jc == 0, jc == N_JC - 1
                            for hh in range(2):
                                nc.tensor.matmul(
                                    pxv[hh][0:65, :],
                                    vnatb[jc][:, 65 * hh : 65 * hh + 65],
                                    pt[hh][:], start=first, stop=last)
                        for hh in range(2):
                            rDf = rp.tile([128, IT], F32, tag="rDf")
                            nc.vector.reciprocal_approx_fast(
                                rDf[0:65, :], pxv[hh][0:65, :])
                            rD = rp.tile([128, IT], F32R, tag="rD")
                            nc.vector.tensor_copy(rD[0:1, :], rDf[64:65, :])
                            rDb = psa.tile([128, IT], F32, tag="rdb", bufs=1, name="rDb")
                            nc.tensor.matmul(rDb[0:64, :], ones_col[:, 0:64],
                                             rD[0:1, :], start=True, stop=True)
                            rDs = rp.tile([128, IT], F32, tag="rDs")
                            nc.vector.tensor_copy(rDs[0:64, :], rDb[0:64, :])
                            nc.vector.tensor_tensor(
                                xvT[b][64 * hh : 64 * hh + 64, isl],
                                pxv[hh][0:64, :], rDs[0:64, :], ALU.mult)

            # ---------- A2A ----------
            a2a_in = dram.tile([NCORES * 128, 512], F32)
            a2a_out = dram.tile([NCORES * 128, 512], F32)
            for j in range(NCORES):
                bj, blkj = j // 4, j % 4
                nc.sync.dma_start(
                    a2a_in[128 * j : 128 * j + 128, :],
                    xvT[bj][:, 512 * blkj : 512 * blkj + 512])
            if sim:
                # timing stand-in for TimelineSim (no collective support):
                # same-size DRAM->DRAM copy
                nc.sync.dma_start(a2a_out[:], a2a_in[:])
            else:
                nc.gpsimd.collective_compute(
                    "AllToAll", ALU.bypass,
                    replica_groups=[list(range(NCORES))],
                    ins=[a2a_in.opt()], outs=[a2a_out.opt()])

            # ---------- out-projection + layernorm ----------
            with (
                tc.tile_pool(name="wout_pool", bufs=1) as wp,
                tc.tile_pool(name="z_pool", bufs=1) as zp,
                tc.tile_pool(name="pso", bufs=2, space="PSUM") as pso,
                tc.tile_pool(name="ln_pool", bufs=2) as lnp,
            ):
                wout = wp.tile([128, N_EC, D], F32R, name="wout_sb")
                wout_src = wout_d.ap().rearrange("(c p) n -> p c n", p=128)
                for e in range(N_EC):
                    nc.sync.dma_start(wout[:, e, :], wout_src[:, e, :])
                bout = wp.tile([1, D], F32R, name="bout_sb")
                gamma = wp.tile([1, D], F32R, name="gamma_sb")
                beta = wp.tile([1, D], F32R, name="beta_sb")
                nc.sync.dma_start(bout[:], bout_d.ap())
                nc.sync.dma_start(gamma[:], gamma_d.ap())
                nc.sync.dma_start(beta[:], beta_d.ap())
                gbc = wp.tile([128, D], F32, name="gb_sb")
                bbc = wp.tile([128, D], F32, name="bb_sb")
                for half in range(2):
                    sl = slice(512 * half, 512 * half + 512)
                    pbc = pso.tile([128, 512], F32, tag="py", bufs=4)
                    nc.tensor.matmul(pbc[:], ones_col[:], gamma[:, sl],
                                     start=True, stop=True)
                    nc.scalar.copy(gbc[:, sl], pbc[:])
                    pbc2 = pso.tile([128, 512], F32, tag="py", bufs=4)
                    nc.tensor.matmul(pbc2[:], ones_col[:], beta[:, sl],
                                     start=True, stop=True)
                    nc.scalar.copy(bbc[:, sl], pbc2[:])

                eps_sb = zp.tile([128, 1], F32, name="eps_sb")
                nc.vector.memset(eps_sb[:], 1e-5)
                xres = [zp.tile([128, 512], F32, name=f"xres_{e}") for e in range(N_EC)]
                zT = [zp.tile([128, 512], F32R, name=f"zT_{e}") for e in range(N_EC)]
                for e in range(N_EC):
                    esl = slice(128 * e, 128 * e + 128)
                    nc.sync.dma_start(xres[e][:], xres_d.ap()[esl, :])
                    nc.sync.dma_start(zT[e][:], a2a_out[esl, :].bitcast(F32R))
                    nc.vector.tensor_tensor(
                        zT[e][:], zT[e][:].bitcast(F32), xres[e][:], ALU.add)

                for ic in range(4):
                    icl = slice(128 * ic, 128 * ic + 128)
                    py = [pso.tile([128, 512], F32, tag="py", bufs=4, name=f"py{_h}") for _h in range(2)]
                    for nh in range(2):
                        nsl = slice(512 * nh, 512 * nh + 512)
                        for e in range(N_EC):
                            nc.tensor.matmul(py[nh][:], zT[e][:, icl],
                                             wout[:, e, nsl],
                                             start=(e == 0), stop=False)
                        nc.tensor.matmul(py[nh][:], ones_col[:], bout[:, nsl],
                                         start=False, stop=True)
                    bn6 = lnp.tile([128, 2, 6], F32, tag="bn6")
                    nc.vector.bn_stats(bn6[:, 0, :], py[0][:])
                    nc.vector.bn_stats(bn6[:, 1, :], py[1][:])
                    bn2 = lnp.tile([128, 2], F32, tag="bn2")
                    nc.vector.bn_aggr(bn2[:], bn6[:])
                    lnv = lnp.tile([128, 1], F32, tag="lnv")
                    nc.scalar.activation(lnv[:], bn2[:, 1:2], AF.Ln, bias=eps_sb[:])
                    rstd = lnp.tile([128, 1], F32, tag="rstd")
                    nc.scalar.activation(rstd[:], lnv[:], AF.Exp, scale=-0.5)
                    yn = lnp.tile([128, D], F32, tag="yn")
                    for nh in range(2):
                        nsl = slice(512 * nh, 512 * nh + 512)
                        t = lnp.tile([128, 512], F32, tag="lt")
                        nc.vector.tensor_scalar(
                            t[:], py[nh][:], bn2[:, 0:1], rstd[:],
                            ALU.subtract, ALU.mult)
                        t2 = lnp.tile([128, 512], F32, tag="lt2")
                        nc.vector.tensor_tensor(t2[:], t[:], gbc[:, nsl], ALU.mult)
                        nc.vector.tensor_tensor(yn[:, nsl], t2[:], bbc[:, nsl], ALU.add)
                    nc.sync.dma_start(y_d.ap()[icl, :], yn[:])

    nc.compile()
    return nc


_NC_CACHE = None


def _get_nc():
    global _NC_CACHE
    if _NC_CACHE is None:
        _NC_CACHE = _build()
    return _NC_CACHE


def _prepare_in_maps(x, w_qkv, b_qkv, w_out, b_out, ln_gamma, ln_beta):
    x = np.asarray(x, dtype=np.float32)
    w_qkv = np.asarray(w_qkv, dtype=np.float32)
    b_qkv = np.asarray(b_qkv, dtype=np.float32)
    w_out = np.ascontiguousarray(np.asarray(w_out, dtype=np.float32))
    b_out = np.asarray(b_out, dtype=np.float32)
    ln_gamma = np.asarray(ln_gamma, dtype=np.float32)
    ln_beta = np.asarray(ln_beta, dtype=np.float32)

    cosM, sinM, SpermT = _rope_consts()
    ident = np.eye(128, dtype=np.float32)
    xT = [np.ascontiguousarray(x[b].T) for b in range(B)]

    in_maps = []
    for c in range(NCORES):
        h0 = 2 * c
        col = slice(HD * h0, HD * h0 + 128)
        myb, myblk = c // 4, c % 4
        m = {
            "xT0": xT[0], "xT1": xT[1],
            "wq": np.ascontiguousarray(w_qkv[:, col]),
            "wk": np.ascontiguousarray(w_qkv[:, D:][:, col]),
            "wv": np.ascontiguousarray(w_qkv[:, 2 * D:][:, col]),
            "bq": np.ascontiguousarray(b_qkv[col])[None, :],
            "bk": np.ascontiguousarray(b_qkv[D:][col])[None, :],
            "bv": np.ascontiguousarray(b_qkv[2 * D:][col])[None, :],
            "wout": w_out,
            "bout": b_out[None, :],
            "gamma": ln_gamma[None, :].astype(np.float32),
            "beta": ln_beta[None, :].astype(np.float32),
            "cosM": cosM, "sinM": sinM, "SpermT": SpermT, "ident": ident,
            "xres": np.ascontiguousarray(xT[myb][:, 512 * myblk : 512 * myblk + 512]),
        }
        in_maps.append(m)
    return in_maps


def _assemble(results):
    out = np.zeros((B, S, D), dtype=np.float32)
    for c in range(NCORES):
        myb, myblk = c // 4, c % 4
        out[myb, 512 * myblk : 512 * myblk + 512, :] = results[c]["y_out"]
    return out


def run(trace=False, **inputs):
    """Full run returning (output, BassKernelResults) — used by test.py for
    profiling; kernel() below is the graded entry point."""
    in_maps = _prepare_in_maps(**inputs)
    res = bass_utils.run_bass_kernel_spmd(
        _get_nc(), in_maps, core_ids=list(range(NCORES)), trace=trace)
    return _assemble(res.results), res


def kernel(**inputs):
    out, _ = run(trace=False, **inputs)
    return out



# revision 34
# speedup vs baseline: 1.1291x; 1.1291x over previous
"""Multi-head rotary attention block on 8 Trainium2 NeuronCores.

Sharding (data-parallel over batch x tensor-parallel over heads):
  core c: batch b = c//4, head group g = c%4 -> heads 4g..4g+3.
  Each core loads only its batch's x, projects q/k/v for its 4 heads,
  runs attention locally, then a 4-way AllToAll inside each batch quad
  redistributes the attention output from head-sharded to token-sharded form
  for the output projection + layernorm.

The AllToAll is chunked: after each 512-token attention tile completes, one
[1024, 128]-per-core exchange fires and that 128-token slice's output
projection + layernorm runs overlapped with the next attention tile, so the
collective+projection tail is almost fully hidden.

Matmuls run as float32r (full-rate fp32 PE mode, self-loading weights - the
bf16 path would split every matmul into LDWEIGHTS+MATMUL pairs and saturate
the PE sequencer). Softmax is exp(s/32) on ScalarE with denominators
accumulated through an extra ones-column in v, divided out via a fast DVE
reciprocal + PE broadcast. Rope is applied as raw*cos + (SpermT^T raw)*sin
with the rotation permutation as a single matmul per tile; qkv biases ride
the PSUM->SBUF copies as tensor_scalar adds instead of extra matmuls.
"""
import numpy as np
import ml_dtypes

import concourse.bass as bass
import concourse.bacc as bacc
import concourse.tile as tile
import concourse.mybir as mybir
from concourse import bass_utils

F32 = mybir.dt.float32
F32R = mybir.dt.float32r
BF16 = mybir.dt.bfloat16
AF = mybir.ActivationFunctionType
ALU = mybir.AluOpType
BF = ml_dtypes.bfloat16

NCORES = 8
B, S, D = 2, 2048, 1024
HEADS, HD = 16, 64
SCALE = 1.0 / float(np.sqrt(D))  # reference scales by full D, not head_dim
IT = 512          # i-tile width for attention / token block
N_IT = S // IT    # 4
JC = 128          # j-chunk
N_JC = S // JC    # 16
N_EC = D // 128   # 8 e-chunks


def _rope_consts():
    rot = HD // 2
    inv_freq = 1.0 / (10000.0 ** (np.arange(0, rot, 2, dtype=np.float64) / rot))
    ang = np.arange(S, dtype=np.float64)[:, None] * inv_freq[None, :]
    ang = np.repeat(ang, 2, axis=-1)  # [S, 32]
    cos, sin = np.cos(ang), np.sin(ang)
    cosM = np.ones((128, S), dtype=np.float32)
    sinM = np.zeros((128, S), dtype=np.float32)
    for base in (0, 64):
        cosM[base : base + 32, :] = cos.T.astype(np.float32)
        sinM[base : base + 32, :] = sin.T.astype(np.float32)
    Sp = np.zeros((128, 128), dtype=np.float32)
    for base in (0, 64):
        for m in range(32):
            r0 = base + m
            if m % 2 == 0:
                Sp[r0, r0 + 1] = -1.0
            else:
                Sp[r0, r0 - 1] = 1.0
    SpermT = np.ascontiguousarray(Sp.T)
    return cosM, sinM, SpermT


def _build(sim=False):
    nc = bacc.Bacc("TRN2", target_bir_lowering=False, debug=False,
                   num_devices=NCORES)

    xT_d = nc.dram_tensor("xT", [D, S], BF16, kind="ExternalInput")
    wq_d = nc.dram_tensor("wq", [D, 256], BF16, kind="ExternalInput")
    wk_d = nc.dram_tensor("wk", [D, 256], BF16, kind="ExternalInput")
    wv_d = nc.dram_tensor("wv", [D, 256], BF16, kind="ExternalInput")
    bqkv_d = nc.dram_tensor("bqkv", [128, 6], F32, kind="ExternalInput")
    wout_d = nc.dram_tensor("wout", [D, D], BF16, kind="ExternalInput")
    bout_d = nc.dram_tensor("bout", [1, D], F32R, kind="ExternalInput")
    gamma_d = nc.dram_tensor("gamma", [1, D], F32R, kind="ExternalInput")
    beta_d = nc.dram_tensor("beta", [1, D], F32R, kind="ExternalInput")
    cosM_d = nc.dram_tensor("cosM", [128, S], BF16, kind="ExternalInput")
    sinM_d = nc.dram_tensor("sinM", [128, S], BF16, kind="ExternalInput")
    spt_d = nc.dram_tensor("SpermT", [128, 128], F32R, kind="ExternalInput")
    ident_d = nc.dram_tensor("ident", [128, 128], F32R, kind="ExternalInput")
    xres_d = nc.dram_tensor("xres", [D, 512], BF16, kind="ExternalInput")
    y_d = nc.dram_tensor("y_out", [512, D], BF16, kind="ExternalOutput")

    groups = [list(range(NCORES))]

    with tile.TileContext(nc) as tc:
        with (
            tc.tile_pool(name="persist", bufs=1) as pp,
            tc.tile_pool(name="dram", bufs=1, space="DRAM") as dram,
            tc.tile_pool(name="ps", bufs=4, space="PSUM") as ps,
            tc.tile_pool(name="psacc", bufs=2, space="PSUM") as psacc,
            tc.tile_pool(name="wk", bufs=1) as wkp,
        ):
            # ---------------- input DMAs (priority ~ emission order) -------
            wq = pp.tile([128, N_EC, 256], BF16, name="wq_sb")
            wk = pp.tile([128, N_EC, 256], BF16, name="wk_sb")
            wv = pp.tile([128, N_EC, 256], BF16, name="wv_sb")
            cosM = pp.tile([128, S], BF16, name="cosM_sb")
            sinM = pp.tile([128, S], BF16, name="sinM_sb")
            spt = pp.tile([128, 128], F32R, name="spt_sb")
            ident = pp.tile([128, 128], F32R, name="ident_sb")
            bqkv = pp.tile([128, 6], F32, name="bqkv_sb")
            wup_src = pp.tile([1, 512], F32R, name="wup_src")
            nc.vector.memset(wup_src[:].bitcast(F32), 0.125)
            xt = pp.tile([128, N_EC, S], BF16, name="xt_sb")
            xt_src = xT_d.ap().rearrange("(c p) s -> p c s", p=128)

            def w_src(w_dd):
                return w_dd.ap().rearrange("(c p) m -> p c m", p=128)

            # first projection unit (k, pc0, it0) gated only by these:
            nc.sync.dma_start(wk[:], w_src(wk_d))
            nc.sync.dma_start(spt[:], spt_d.ap())
            nc.sync.dma_start(bqkv[:], bqkv_d.ap())
            nc.sync.dma_start(cosM[:], cosM_d.ap())
            nc.sync.dma_start(sinM[:], sinM_d.ap())
            nc.sync.dma_start(xt[:, 0:4, 0:IT], xt_src[:, 0:4, 0:IT])
            nc.sync.dma_start(xt[:, 4:8, 0:IT], xt_src[:, 4:8, 0:IT])
            nc.sync.dma_start(wq[:], w_src(wq_d))
            nc.sync.dma_start(wv[:], w_src(wv_d))
            nc.sync.dma_start(ident[:], ident_d.ap())
            for it in range(1, N_IT):
                isl = slice(IT * it, IT * it + IT)
                nc.sync.dma_start(xt[:, :, isl], xt_src[:, :, isl])

            ones_bf = pp.tile([1, 128], F32R, name="ones_bf")
            nc.vector.memset(ones_bf[:].bitcast(F32), 1.0)
            ones_fr = pp.tile([1, 64], F32R, name="ones_fr")
            nc.vector.memset(ones_fr[:].bitcast(F32), 1.0)
            # PE warmup: dep-free matmuls fill the DMA-bound prolog so the
            # tensor engine reaches full clock before real work arrives
            wup = ps.tile([128, 512], F32, tag="sc", name="wup")
            for _ in range(18):
                nc.tensor.matmul(wup[:], ones_bf[:], wup_src[:],
                                 start=True, stop=True)
            eps_sb = pp.tile([128, 1], F32, name="eps_sb")
            nc.vector.memset(eps_sb[:], 1e-5)

            kT = pp.tile([128, 2, S], F32R, name="kT_sb")
            vnat = [pp.tile([128, 260], F32R, name=f"vnat_{j}")
                    for j in range(N_JC)]
            for j in range(N_JC):
                nc.vector.memset(vnat[j][:, 64::65].bitcast(F32), 1.0)
            xvT = pp.tile([128, 2, S], BF16, name="xvT_sb")

            # ---------------- projection helpers ---------------------------
            # Units are split into phase1 (PSUM accumulation + bias copy) and
            # phase2 (rope / transposes). One phase2 stays pending so the next
            # unit's matmuls fill the PE while DVE finishes the previous
            # unit's bias add - the in-order PE queue never waits on DVE.
            unit_pipe = []

            def pump_units(f2=None):
                while unit_pipe:
                    unit_pipe.pop(0)()
                if f2 is not None:
                    unit_pipe.append(f2)

            def emit_qk_unit(dst_ap, w_sb, bcol, pc, it):
                # dst_ap: [128, 512] destination (bf16 kT slice or f32r q tile)
                isl = slice(IT * it, IT * it + IT)
                praw = ps.tile([128, IT], F32, tag="sc", name="praw")
                for e in range(N_EC):
                    nc.tensor.matmul(praw[:],
                                     w_sb[:, e, 128 * pc : 128 * pc + 128],
                                     xt[:, e, isl],
                                     start=(e == 0), stop=(e == N_EC - 1))
                raw = wkp.tile([128, IT], F32R, tag="raw", bufs=2, name="raw")
                nc.vector.tensor_scalar(raw[:], praw[:],
                                        bqkv[:, bcol : bcol + 1], None, ALU.add)

                def phase2(dst_ap=dst_ap):
                    prot = ps.tile([128, IT], F32, tag="sc", name="prot")
                    nc.tensor.matmul(prot[:], spt[:], raw[:],
                                     start=True, stop=True)
                    t1 = wkp.tile([128, IT], BF16, tag="t1", bufs=2,
                                  name="t1")
                    nc.vector.tensor_tensor(t1[:], prot[:], sinM[:, isl],
                                            ALU.mult)
                    t2 = wkp.tile([128, IT], BF16, tag="t2", bufs=2,
                                  name="t2")
                    nc.gpsimd.tensor_tensor(t2[:], raw[:].bitcast(F32),
                                            cosM[:, isl], ALU.mult)
                    nc.vector.tensor_tensor(dst_ap, t1[:], t2[:],
                                            ALU.add)

                pump_units(phase2)

            def emit_q_unit(pc, it):
                q_t = wkp.tile([128, IT], F32R, tag="qt", bufs=3, name="q_t")
                emit_qk_unit(q_t[:], wq, pc, pc, it)
                return q_t

            def emit_v_unit(pc, it):
                isl = slice(IT * it, IT * it + IT)
                pvt = ps.tile([128, IT], F32, tag="sc", name="pvt")
                for e in range(N_EC):
                    nc.tensor.matmul(pvt[:],
                                     wv[:, e, 128 * pc : 128 * pc + 128],
                                     xt[:, e, isl],
                                     start=(e == 0), stop=(e == N_EC - 1))
                vt = wkp.tile([128, IT], F32R, tag="vt", bufs=2, name="vt")
                nc.vector.tensor_scalar(vt[:], pvt[:],
                                        bqkv[:, 4 + pc : 5 + pc], None, ALU.add)

                def phase2():
                    for jj in range(IT // JC):
                        jcc = it * (IT // JC) + jj
                        ptr = ps.tile([128, 128], F32R, tag="sc", name="ptr")
                        nc.tensor.transpose(
                            ptr[:], vt[:, JC * jj : JC * jj + JC], ident[:])
                        for hh in range(2):
                            h = 2 * pc + hh
                            nc.vector.tensor_copy(
                                vnat[jcc][:, 65 * h : 65 * h + 64],
                                ptr[:, 64 * hh : 64 * hh + 64].bitcast(F32))

                pump_units(phase2)

            # ---------------- attention helper ------------------------------
            def emit_attention(it, pc, q_t, fillers=None):
                pump_units()
                isl = slice(IT * it, IT * it + IT)
                pxv = psacc.tile([128, 1024], F32, tag="acc", name="pxv")
                for jc in range(N_JC):
                    pump_units()  # pending phase2 lands 1 j-chunk after its
                    if fillers and jc in fillers:  # phase1 - always in time
                        for f in fillers[jc]:
                            f()
                    jsl = slice(JC * jc, JC * jc + JC)
                    for hh in range(2):
                        h = 2 * pc + hh
                        hsl = slice(64 * hh, 64 * hh + 64)
                        psc = ps.tile([128, IT], F32, tag="sc", name="psc")
                        nc.tensor.matmul(psc[:], kT[hsl, pc, jsl],
                                         q_t[hsl, :],
                                         start=True, stop=True)
                        pt = wkp.tile([128, IT], F32R, tag="pt", bufs=4,
                                      name="pt")
                        nc.scalar.activation(pt[:], psc[:], AF.Exp, scale=SCALE)
                        nc.tensor.matmul(
                            pxv[0:65, 512 * hh : 512 * hh + 512],
                            vnat[jc][:, 65 * h : 65 * h + 65],
                            pt[:], start=(jc == 0), stop=(jc == N_JC - 1))
                # softmax denominator reciprocal (broadcast + multiply are
                # deferred into the next tile-group's fillers so their
                # dep-stalls never block the in-order PE queue)
                rDf = wkp.tile([1, 1024], F32, tag="rdf", bufs=1, name="rDf")
                nc.vector.reciprocal_approx_fast(rDf[:], pxv[64:65, :])
                rD = wkp.tile([1, 1024], F32R, tag="rd", bufs=1, name="rD")
                nc.gpsimd.tensor_copy(rD[:], rDf[:])
                return pxv, rD

            def emit_divide(it, pc, pxv, rD):
                isl = slice(IT * it, IT * it + IT)
                for hh in range(2):
                    nsl = slice(512 * hh, 512 * hh + 512)
                    rDb = ps.tile([128, IT], F32, tag="sc", name="rDb")
                    nc.tensor.matmul(rDb[0:64, :], ones_fr[:], rD[:, nsl],
                                     start=True, stop=True)
                    rDs = wkp.tile([64, IT], BF16, tag="rds", bufs=2,
                                   name="rDs")
                    nc.vector.tensor_copy(rDs[:], rDb[0:64, :])
                    nc.vector.tensor_tensor(
                        xvT[64 * hh : 64 * hh + 64, pc, isl],
                        pxv[0:64, nsl], rDs[:], ALU.mult)

            # ---------------- emit: first units, rest via fillers -----------
            def emit_k_unit(pc, it):
                emit_qk_unit(kT[:, pc, IT * it : IT * it + IT],
                             wk, 2 + pc, pc, it)

            emit_k_unit(0, 0)
            emit_v_unit(0, 0)
            q_next = [emit_q_unit(0, 0), None]

            # late-phase inputs: DMAs emitted early (low queue priority is
            # fine - only out-projection needs them), broadcast matmuls
            # deferred into an it0 filler so they never stall the PE queue.
            wout = pp.tile([128, N_EC, D], BF16, name="wout_sb")
            bout = pp.tile([1, D], F32R, name="bout_sb")
            gamma = pp.tile([1, D], F32R, name="gamma_sb")
            beta = pp.tile([1, D], F32R, name="beta_sb")
            xres = pp.tile([128, N_EC, 512], BF16, name="xres_sb")
            gbc = pp.tile([128, D], BF16, name="gbc_sb")
            bbc = pp.tile([128, D], BF16, name="bbc_sb")
            nc.sync.dma_start(
                wout[:], wout_d.ap().rearrange("(c p) n -> p c n", p=128))
            nc.sync.dma_start(bout[:], bout_d.ap())
            nc.sync.dma_start(gamma[:], gamma_d.ap())
            nc.sync.dma_start(beta[:], beta_d.ap())
            nc.sync.dma_start(
                xres[:], xres_d.ap().rearrange("(c p) s -> p c s", p=128))

            def emit_gb_bcast():
                for src_t, dst in ((gamma, gbc), (beta, bbc)):
                    for half in range(2):
                        nsl = slice(512 * half, 512 * half + 512)
                        pbc = ps.tile([128, 512], F32, tag="sc", name="pbc")
                        nc.tensor.matmul(pbc[:], ones_bf[:], src_t[:, nsl],
                                         start=True, stop=True)
                        nc.vector.tensor_copy(dst[:, nsl], pbc[:])

            # 8-way exchange: sender block j = [256 chan, 64 tok] slice j
            # of its batch; receiver j gets batch-0 channels (senders 0-3)
            # in rows 0-1023 and batch-1 (senders 4-7) in rows 1024-2047.
            a2a_in = [dram.tile([2048, 64], BF16, name=f"a2a_in{k}")
                      for k in range(N_IT)]
            a2a_out = [dram.tile([2048, 64], BF16, name=f"a2a_out{k}")
                       for k in range(N_IT)]

            # ---------------- out-projection + layernorm chunk --------------
            def emit_outproj_stages(k, half=None):
                """Out-projection for token block k as a list of (slot, fn)
                emissions so the zk DMA latency and the matmul burst spread
                over several j-chunks instead of stalling the PE queue."""
                ki = k if half is None else 3 + half
                tw = 64 if half is None else 32
                nt = 2 * tw
                zk = wkp.tile([128, N_EC, nt], BF16, tag="zk", bufs=2,
                              name="zk")
                py = [ps.tile([nt, 512], F32, tag="sc", name=f"py{nh}")
                      for nh in range(2)]

                def s_load():
                    zsrc = a2a_out[ki].rearrange("(b e p) t -> p b e t",
                                                 p=128, b=2, e=N_EC)
                    for bh in range(2):
                        tsl = slice(tw * bh, tw * bh + tw)
                        xoff = JC * k + 64 * bh + (32 * half if half else 0)
                        nc.sync.dma_start(zk[:, :, tsl], zsrc[:, bh])
                        nc.gpsimd.tensor_tensor(
                            zk[:, :, tsl], zk[:, :, tsl],
                            xres[:, :, xoff : xoff + tw], ALU.add)

                def s_mm(nh):
                    nsl = slice(512 * nh, 512 * nh + 512)
                    for e in range(N_EC):
                        nc.tensor.matmul(py[nh][:], zk[:, e, :],
                                         wout[:, e, nsl],
                                         start=(e == 0), stop=False)
                    nc.tensor.matmul(py[nh][:], ones_bf[:, 0:nt],
                                     bout[:, nsl], start=False, stop=True)

                def s_ln():
                    emit_ln_store(k, py, half)

                return [s_load, lambda: s_mm(0), lambda: s_mm(1), s_ln]

            def emit_ln_store(k, py, half=None):
                tw = 64 if half is None else 32
                nt = 2 * tw
                bn6 = wkp.tile([128, 2, 6], F32, tag="bn6", bufs=2, name="bn6")
                nc.vector.bn_stats(bn6[0:nt, 0, :], py[0][:])
                nc.vector.bn_stats(bn6[0:nt, 1, :], py[1][:])
                bn2 = wkp.tile([128, 2], F32, tag="bn2", bufs=2, name="bn2")
                nc.vector.bn_aggr(bn2[0:nt], bn6[0:nt])
                # rstd = (var+eps)^-0.5 without Ln (keeps ScalarE on the Exp
                # table the whole kernel): Mitchell bitwise log2 on DVE ->
                # exp(-0.5 ln v) seed -> one Newton step to 3e-4 accuracy.
                vv = wkp.tile([128, 1], F32, tag="lnv", bufs=2, name="vv")
                nc.vector.tensor_scalar(vv[0:nt], bn2[0:nt, 1:2], 1e-5, None,
                                        ALU.add)
                iv = wkp.tile([128, 1], F32, tag="iv", bufs=2, name="iv")
                nc.vector.tensor_copy(iv[0:nt], vv[0:nt].bitcast(mybir.dt.int32))
                lnv = wkp.tile([128, 1], F32, tag="lnv2", bufs=2, name="lnv")
                LN2 = float(np.log(2.0))
                nc.vector.tensor_scalar(lnv[0:nt], iv[0:nt], LN2 / (1 << 23),
                                        -(127.0 - 0.0450) * LN2,
                                        ALU.mult, ALU.add)
                r0 = wkp.tile([128, 1], F32, tag="rstd0", bufs=2, name="r0")
                nc.scalar.activation(r0[0:nt], lnv[0:nt], AF.Exp, scale=-0.5)
                r2 = wkp.tile([128, 1], F32, tag="r2", bufs=2, name="r2")
                nc.vector.tensor_tensor(r2[0:nt], r0[0:nt], r0[0:nt], ALU.mult)
                nc.vector.tensor_tensor(r2[0:nt], r2[0:nt], vv[0:nt], ALU.mult)
                nc.vector.tensor_scalar(r2[0:nt], r2[0:nt], -0.5, 1.5,
                                        ALU.mult, ALU.add)
                rstd = wkp.tile([128, 1], F32, tag="rstd", bufs=2, name="rstd")
                nc.vector.tensor_tensor(rstd[0:nt], r0[0:nt], r2[0:nt], ALU.mult)
                yn = wkp.tile([128, D], BF16, tag="yn", bufs=2, name="yn")
                for nh in range(2):
                    nsl = slice(512 * nh, 512 * nh + 512)
                    t = wkp.tile([128, 512], BF16, tag="lt", bufs=2,
                                 name="lt")
                    nc.vector.tensor_scalar(t[0:nt], py[nh][:],
                                            bn2[0:nt, 0:1], rstd[0:nt],
                                            ALU.subtract, ALU.mult)
                    t2 = wkp.tile([128, 512], BF16, tag="lt2", bufs=2,
                                  name="lt2")
                    nc.vector.tensor_tensor(t2[0:nt], t[0:nt], gbc[0:nt, nsl],
                                            ALU.mult)
                    nc.vector.tensor_tensor(yn[0:nt, nsl], t2[0:nt],
                                            bbc[0:nt, nsl], ALU.add)
                if half is None:
                    nc.sync.dma_start(y_d.ap()[JC * k : JC * k + JC, :],
                                      yn[:])
                else:
                    # half h covers 32-token slices of both batch halves
                    ydst = y_d.ap()[JC * k : JC * k + JC, :].rearrange(
                        "(b t) n -> b t n", b=2)
                    nc.sync.dma_start(
                        ydst[:, 32 * half : 32 * half + 32, :],
                        yn[0:nt, :].rearrange("(b t) n -> b t n", b=2))

            # ---------------- main loop -------------------------------------
            def mkf(fn, *args):
                return lambda: fn(*args)

            pend = []  # deferred divide-epilogues: (it, pc, pxv, rD)

            def drain_divides():
                out = [mkf(emit_divide, *args) for args in pend]
                pend.clear()
                return out

            def emit_stage_a2a(it, half=None):
                ki = it if half is None else 3 + half
                tw = 64 if half is None else 32
                off = 0 if not half else 32
                a2a_dst = a2a_in[ki].rearrange("(j c p) t -> p c j t",
                                               p=128, c=2, j=8)
                src_ap = xvT[:, :, IT * it : IT * it + IT].rearrange(
                    "p c (j t) -> p c j t", j=8)
                for pc in range(2):
                    nc.sync.dma_start(a2a_dst[:, pc],
                                      src_ap[:, pc, :, off : off + tw])
                if sim:
                    # timing stand-in for TimelineSim (no collective support)
                    nc.sync.dma_start(a2a_out[ki][:], a2a_in[ki][:])
                else:
                    nc.gpsimd.collective_compute(
                        "AllToAll", ALU.bypass,
                        replica_groups=groups,
                        ins=[a2a_in[ki].opt()], outs=[a2a_out[ki].opt()])

            for it in range(N_IT):
                q0, q1 = q_next
                nq = [None, None]

                def grab0(itn):
                    nq[0] = emit_q_unit(0, itn)

                def grab1(itn):
                    nq[1] = emit_q_unit(1, itn)

                if it == 0:
                    # prime everything else under the it0 windows; pc0 only
                    # needs v(0,*) (vnat subtiles for heads 0-1), so v(1,*)
                    # rides the pc1 window
                    f0 = {
                        1: [mkf(emit_k_unit, 0, 1)],
                        3: [mkf(emit_v_unit, 0, 1)],
                        5: [mkf(emit_k_unit, 0, 2)],
                        7: [mkf(emit_v_unit, 0, 2)],
                        9: [mkf(emit_k_unit, 0, 3)],
                        11: [mkf(emit_v_unit, 0, 3)],
                        13: [mkf(emit_k_unit, 1, 0)],
                        14: [mkf(emit_v_unit, 1, 0)],
                    }
                    pxv, rD = emit_attention(0, 0, q0, f0)
                    pend.append((0, 0, pxv, rD))
                    q1 = emit_q_unit(1, 0)
                    f1 = {1: [mkf(emit_k_unit, 1, 1)],
                          4: drain_divides(),
                          3: [mkf(emit_v_unit, 1, 1)],
                          5: [mkf(emit_k_unit, 1, 2)],
                          6: [emit_gb_bcast],
                          7: [mkf(emit_v_unit, 1, 2)],
                          8: [mkf(grab0, 1)],
                          9: [mkf(emit_k_unit, 1, 3)],
                          11: [mkf(emit_v_unit, 1, 3)],
                          12: [mkf(grab1, 1)]}
                    pxv, rD = emit_attention(0, 1, q1, f1)
                    pend.append((0, 1, pxv, rD))
                else:
                    # drain prev divide, then exchange + out-project the
                    # previous token block spread over this iteration
                    f0 = {4: drain_divides()
                          + [mkf(emit_stage_a2a, it - 1)]}
                    pxv, rD = emit_attention(it, 0, q0, f0)
                    pend.append((it, 0, pxv, rD))
                    st = emit_outproj_stages(it - 1)
                    f1 = {4: drain_divides() + [st[0]],
                          6: [st[1]], 9: [st[2]], 12: [st[3]]}
                    if it < N_IT - 1:
                        f1[8] = [mkf(grab0, it + 1)]
                        f1[13] = [mkf(grab1, it + 1)]
                    pxv, rD = emit_attention(it, 1, q1, f1)
                    pend.append((it, 1, pxv, rD))
                q_next = nq
            for f in drain_divides():
                f()
            emit_stage_a2a(N_IT - 1, half=0)
            emit_stage_a2a(N_IT - 1, half=1)
            sa = emit_outproj_stages(N_IT - 1, half=0)
            sb = emit_outproj_stages(N_IT - 1, half=1)
            for s in (sa[0], sb[0], sa[1], sa[2], sb[1], sb[2], sa[3], sb[3]):
                s()

    nc.compile()
    return nc


_NC_CACHE = None


def _get_nc():
    global _NC_CACHE
    if _NC_CACHE is None:
        _NC_CACHE = _build()
    return _NC_CACHE


def _prepare_in_maps(x, w_qkv, b_qkv, w_out, b_out, ln_gamma, ln_beta):
    x = np.asarray(x, dtype=np.float32)
    w_qkv = np.asarray(w_qkv, dtype=np.float32)
    b_qkv = np.asarray(b_qkv, dtype=np.float32)
    w_out = np.ascontiguousarray(np.asarray(w_out, dtype=np.float32))
    b_out = np.asarray(b_out, dtype=np.float32)
    ln_gamma = np.asarray(ln_gamma, dtype=np.float32)
    ln_beta = np.asarray(ln_beta, dtype=np.float32)

    cosM, sinM, SpermT = _rope_consts()
    ident = np.eye(128, dtype=np.float32)
    xT = [np.ascontiguousarray(x[b].T) for b in range(B)]

    in_maps = []
    for c in range(NCORES):
        b, g = c // 4, c % 4
        col = slice(256 * g, 256 * g + 256)
        bq = b_qkv[col]
        bk = b_qkv[D:][col]
        bv = b_qkv[2 * D:][col]
        bqkv6 = np.ascontiguousarray(
            np.stack([bq[:128], bq[128:], bk[:128], bk[128:],
                      bv[:128], bv[128:]], axis=1).astype(np.float32))
        # my output tokens: per chunk k, 64 tokens of each batch at
        # 512k + 64c (c = global core id = receiver rank)
        xres = np.concatenate(
            [xT[bb][:, IT * k + 64 * c : IT * k + 64 * c + 64]
             for k in range(N_IT) for bb in range(B)], axis=1)
        m = {
            "xT": xT[b].astype(BF),
            "wq": np.ascontiguousarray(w_qkv[:, col]).astype(BF),
            "wk": np.ascontiguousarray(w_qkv[:, D:][:, col]).astype(BF),
            "wv": np.ascontiguousarray(w_qkv[:, 2 * D:][:, col]).astype(BF),
            "bqkv": bqkv6,
            "wout": w_out.astype(BF),
            "bout": b_out[None, :],
            "gamma": ln_gamma[None, :],
            "beta": ln_beta[None, :],
            "cosM": cosM.astype(BF), "sinM": sinM.astype(BF),
            "SpermT": SpermT, "ident": ident,
            "xres": np.ascontiguousarray(xres).astype(BF),
        }
        in_maps.append(m)
    return in_maps


def _assemble(results):
    out = np.zeros((B, S, D), dtype=np.float32)
    for c in range(NCORES):
        y = results[c]["y_out"]
        for k in range(N_IT):
            for bb in range(B):
                t0 = IT * k + 64 * c
                out[bb, t0 : t0 + 64, :] = \
                    y[JC * k + 64 * bb : JC * k + 64 * bb + 64]
    return out


def run(trace=False, **inputs):
    """Full run returning (output, BassKernelResults) — used by test.py for
    profiling; kernel() below is the graded entry point."""
    in_maps = _prepare_in_maps(**inputs)
    res = bass_utils.run_bass_kernel_spmd(
        _get_nc(), in_maps, core_ids=list(range(NCORES)), trace=trace)
    return _assemble(res.results), res


def kernel(**inputs):
    out, _ = run(trace=False, **inputs)
    return out


# revision 36
# speedup vs baseline: 1.1325x; 1.0030x over previous
"""Multi-head rotary attention block on 8 Trainium2 NeuronCores.

Sharding (data-parallel over batch x tensor-parallel over heads):
  core c: batch b = c//4, head group g = c%4 -> heads 4g..4g+3.
  Each core loads only its batch's x, projects q/k/v for its 4 heads,
  runs attention locally, then a 4-way AllToAll inside each batch quad
  redistributes the attention output from head-sharded to token-sharded form
  for the output projection + layernorm.

The AllToAll is chunked: after each 512-token attention tile completes, one
[1024, 128]-per-core exchange fires and that 128-token slice's output
projection + layernorm runs overlapped with the next attention tile, so the
collective+projection tail is almost fully hidden.

Matmuls run as float32r (full-rate fp32 PE mode, self-loading weights - the
bf16 path would split every matmul into LDWEIGHTS+MATMUL pairs and saturate
the PE sequencer). Softmax is exp(s/32) on ScalarE with denominators
accumulated through an extra ones-column in v, divided out via a fast DVE
reciprocal + PE broadcast. Rope is applied as raw*cos + (SpermT^T raw)*sin
with the rotation permutation as a single matmul per tile; qkv biases ride
the PSUM->SBUF copies as tensor_scalar adds instead of extra matmuls.
"""
import numpy as np
import ml_dtypes

import concourse.bass as bass
import concourse.bacc as bacc
import concourse.tile as tile
import concourse.mybir as mybir
from concourse import bass_utils

F32 = mybir.dt.float32
F32R = mybir.dt.float32r
BF16 = mybir.dt.bfloat16
AF = mybir.ActivationFunctionType
ALU = mybir.AluOpType
BF = ml_dtypes.bfloat16

NCORES = 8
B, S, D = 2, 2048, 1024
HEADS, HD = 16, 64
SCALE = 1.0 / float(np.sqrt(D))  # reference scales by full D, not head_dim
IT = 512          # i-tile width for attention / token block
N_IT = S // IT    # 4
JC = 128          # j-chunk
N_JC = S // JC    # 16
N_EC = D // 128   # 8 e-chunks


def _rope_consts():
    rot = HD // 2
    inv_freq = 1.0 / (10000.0 ** (np.arange(0, rot, 2, dtype=np.float64) / rot))
    ang = np.arange(S, dtype=np.float64)[:, None] * inv_freq[None, :]
    ang = np.repeat(ang, 2, axis=-1)  # [S, 32]
    cos, sin = np.cos(ang), np.sin(ang)
    cosM = np.ones((128, S), dtype=np.float32)
    sinM = np.zeros((128, S), dtype=np.float32)
    for base in (0, 64):
        cosM[base : base + 32, :] = cos.T.astype(np.float32)
        sinM[base : base + 32, :] = sin.T.astype(np.float32)
    Sp = np.zeros((128, 128), dtype=np.float32)
    for base in (0, 64):
        for m in range(32):
            r0 = base + m
            if m % 2 == 0:
                Sp[r0, r0 + 1] = -1.0
            else:
                Sp[r0, r0 - 1] = 1.0
    SpermT = np.ascontiguousarray(Sp.T)
    return cosM, sinM, SpermT


def _build(sim=False):
    nc = bacc.Bacc("TRN2", target_bir_lowering=False, debug=False,
                   num_devices=NCORES)

    xT_d = nc.dram_tensor("xT", [D, S], BF16, kind="ExternalInput")
    wq_d = nc.dram_tensor("wq", [D, 256], BF16, kind="ExternalInput")
    wk_d = nc.dram_tensor("wk", [D, 256], BF16, kind="ExternalInput")
    wv_d = nc.dram_tensor("wv", [D, 256], BF16, kind="ExternalInput")
    bqkv_d = nc.dram_tensor("bqkv", [128, 6], F32, kind="ExternalInput")
    wout_d = nc.dram_tensor("wout", [D, D], BF16, kind="ExternalInput")
    bout_d = nc.dram_tensor("bout", [1, D], F32R, kind="ExternalInput")
    gamma_d = nc.dram_tensor("gamma", [1, D], F32R, kind="ExternalInput")
    beta_d = nc.dram_tensor("beta", [1, D], F32R, kind="ExternalInput")
    cosM_d = nc.dram_tensor("cosM", [128, S], BF16, kind="ExternalInput")
    sinM_d = nc.dram_tensor("sinM", [128, S], BF16, kind="ExternalInput")
    spt_d = nc.dram_tensor("SpermT", [128, 128], F32R, kind="ExternalInput")
    ident_d = nc.dram_tensor("ident", [128, 128], F32R, kind="ExternalInput")
    xres_d = nc.dram_tensor("xres", [D, 512], BF16, kind="ExternalInput")
    y_d = nc.dram_tensor("y_out", [512, D], BF16, kind="ExternalOutput")

    groups = [list(range(NCORES))]

    with tile.TileContext(nc) as tc:
        with (
            tc.tile_pool(name="persist", bufs=1) as pp,
            tc.tile_pool(name="dram", bufs=1, space="DRAM") as dram,
            tc.tile_pool(name="ps", bufs=4, space="PSUM") as ps,
            tc.tile_pool(name="psacc", bufs=2, space="PSUM") as psacc,
            tc.tile_pool(name="wk", bufs=1) as wkp,
        ):
            # ---------------- input DMAs (priority ~ emission order) -------
            wq = pp.tile([128, N_EC, 256], BF16, name="wq_sb")
            wk = pp.tile([128, N_EC, 256], BF16, name="wk_sb")
            wv = pp.tile([128, N_EC, 256], BF16, name="wv_sb")
            cosM = pp.tile([128, S], BF16, name="cosM_sb")
            sinM = pp.tile([128, S], BF16, name="sinM_sb")
            spt = pp.tile([128, 128], F32R, name="spt_sb")
            ident = pp.tile([128, 128], F32R, name="ident_sb")
            bqkv = pp.tile([128, 6], F32, name="bqkv_sb")
            wup_src = pp.tile([1, 512], F32R, name="wup_src")
            nc.vector.memset(wup_src[:].bitcast(F32), 0.125)
            xt = pp.tile([128, N_EC, S], BF16, name="xt_sb")
            xt_src = xT_d.ap().rearrange("(c p) s -> p c s", p=128)

            def w_src(w_dd):
                return w_dd.ap().rearrange("(c p) m -> p c m", p=128)

            # first projection unit (k, pc0, it0) gated only by these:
            nc.sync.dma_start(wk[:], w_src(wk_d))
            nc.sync.dma_start(spt[:], spt_d.ap())
            nc.sync.dma_start(bqkv[:], bqkv_d.ap())
            nc.sync.dma_start(cosM[:], cosM_d.ap())
            nc.sync.dma_start(sinM[:], sinM_d.ap())
            nc.sync.dma_start(xt[:, 0:4, 0:IT], xt_src[:, 0:4, 0:IT])
            nc.sync.dma_start(xt[:, 4:8, 0:IT], xt_src[:, 4:8, 0:IT])
            nc.sync.dma_start(wq[:], w_src(wq_d))
            nc.sync.dma_start(wv[:], w_src(wv_d))
            nc.sync.dma_start(ident[:], ident_d.ap())
            for it in range(1, N_IT):
                isl = slice(IT * it, IT * it + IT)
                nc.sync.dma_start(xt[:, :, isl], xt_src[:, :, isl])

            ones_bf = pp.tile([1, 128], F32R, name="ones_bf")
            nc.vector.memset(ones_bf[:].bitcast(F32), 1.0)
            ones_fr = pp.tile([1, 64], F32R, name="ones_fr")
            nc.vector.memset(ones_fr[:].bitcast(F32), 1.0)
            # PE warmup: dep-free matmuls fill the DMA-bound prolog so the
            # tensor engine reaches full clock before real work arrives
            wup = ps.tile([128, 512], F32, tag="sc", name="wup")
            for _ in range(18):
                nc.tensor.matmul(wup[:], ones_bf[:], wup_src[:],
                                 start=True, stop=True)
            eps_sb = pp.tile([128, 1], F32, name="eps_sb")
            nc.vector.memset(eps_sb[:], 1e-5)

            kT = pp.tile([128, 2, S], F32R, name="kT_sb")
            vnat = [pp.tile([128, 260], F32R, name=f"vnat_{j}")
                    for j in range(N_JC)]
            for j in range(N_JC):
                nc.vector.memset(vnat[j][:, 64::65].bitcast(F32), 1.0)
            xvT = pp.tile([128, 2, S], BF16, name="xvT_sb")

            # ---------------- projection helpers ---------------------------
            # Units are split into phase1 (PSUM accumulation + bias copy) and
            # phase2 (rope / transposes). One phase2 stays pending so the next
            # unit's matmuls fill the PE while DVE finishes the previous
            # unit's bias add - the in-order PE queue never waits on DVE.
            unit_pipe = []

            def pump_units(f2=None):
                while unit_pipe:
                    unit_pipe.pop(0)()
                if f2 is not None:
                    unit_pipe.append(f2)

            def emit_qk_unit(dst_ap, w_sb, bcol, pc, it):
                # dst_ap: [128, 512] destination (bf16 kT slice or f32r q tile)
                isl = slice(IT * it, IT * it + IT)
                praw = ps.tile([128, IT], F32, tag="sc", name="praw")
                for e in range(N_EC):
                    nc.tensor.matmul(praw[:],
                                     w_sb[:, e, 128 * pc : 128 * pc + 128],
                                     xt[:, e, isl],
                                     start=(e == 0), stop=(e == N_EC - 1))
                raw = wkp.tile([128, IT], F32R, tag="raw", bufs=2, name="raw")
                nc.vector.tensor_scalar(raw[:], praw[:],
                                        bqkv[:, bcol : bcol + 1], None, ALU.add)

                def phase2(dst_ap=dst_ap):
                    prot = ps.tile([128, IT], F32, tag="sc", name="prot")
                    nc.tensor.matmul(prot[:], spt[:], raw[:],
                                     start=True, stop=True)
                    t1 = wkp.tile([128, IT], BF16, tag="t1", bufs=2,
                                  name="t1")
                    nc.vector.tensor_tensor(t1[:], prot[:], sinM[:, isl],
                                            ALU.mult)
                    t2 = wkp.tile([128, IT], BF16, tag="t2", bufs=2,
                                  name="t2")
                    nc.gpsimd.tensor_tensor(t2[:], raw[:].bitcast(F32),
                                            cosM[:, isl], ALU.mult)
                    nc.vector.tensor_tensor(dst_ap, t1[:], t2[:],
                                            ALU.add)

                pump_units(phase2)

            def emit_q_unit(pc, it):
                q_t = wkp.tile([128, IT], F32R, tag="qt", bufs=3, name="q_t")
                emit_qk_unit(q_t[:], wq, pc, pc, it)
                return q_t

            def emit_v_unit(pc, it):
                isl = slice(IT * it, IT * it + IT)
                pvt = ps.tile([128, IT], F32, tag="sc", name="pvt")
                for e in range(N_EC):
                    nc.tensor.matmul(pvt[:],
                                     wv[:, e, 128 * pc : 128 * pc + 128],
                                     xt[:, e, isl],
                                     start=(e == 0), stop=(e == N_EC - 1))
                vt = wkp.tile([128, IT], F32R, tag="vt", bufs=2, name="vt")
                nc.vector.tensor_scalar(vt[:], pvt[:],
                                        bqkv[:, 4 + pc : 5 + pc], None, ALU.add)

                def phase2():
                    for jj in range(IT // JC):
                        jcc = it * (IT // JC) + jj
                        ptr = ps.tile([128, 128], F32R, tag="sc", name="ptr")
                        nc.tensor.transpose(
                            ptr[:], vt[:, JC * jj : JC * jj + JC], ident[:])
                        for hh in range(2):
                            h = 2 * pc + hh
                            nc.vector.tensor_copy(
                                vnat[jcc][:, 65 * h : 65 * h + 64],
                                ptr[:, 64 * hh : 64 * hh + 64].bitcast(F32))

                pump_units(phase2)

            # ---------------- attention helper ------------------------------
            def emit_attention(it, pc, q_t, fillers=None):
                pump_units()
                isl = slice(IT * it, IT * it + IT)
                pxv = psacc.tile([128, 1024], F32, tag="acc", name="pxv")
                for jc in range(N_JC):
                    pump_units()  # pending phase2 lands 1 j-chunk after its
                    if fillers and jc in fillers:  # phase1 - always in time
                        for f in fillers[jc]:
                            f()
                    jsl = slice(JC * jc, JC * jc + JC)
                    for hh in range(2):
                        h = 2 * pc + hh
                        hsl = slice(64 * hh, 64 * hh + 64)
                        psc = ps.tile([128, IT], F32, tag="sc", name="psc")
                        nc.tensor.matmul(psc[:], kT[hsl, pc, jsl],
                                         q_t[hsl, :],
                                         start=True, stop=True)
                        pt = wkp.tile([128, IT], F32R, tag="pt", bufs=4,
                                      name="pt")
                        nc.scalar.activation(pt[:], psc[:], AF.Exp, scale=SCALE)
                        nc.tensor.matmul(
                            pxv[0:65, 512 * hh : 512 * hh + 512],
                            vnat[jc][:, 65 * h : 65 * h + 65],
                            pt[:], start=(jc == 0), stop=(jc == N_JC - 1))
                # softmax denominator reciprocal (broadcast + multiply are
                # deferred into the next tile-group's fillers so their
                # dep-stalls never block the in-order PE queue)
                rDf = wkp.tile([1, 1024], F32, tag="rdf", bufs=1, name="rDf")
                nc.vector.reciprocal_approx_fast(rDf[:], pxv[64:65, :])
                rD = wkp.tile([1, 1024], F32R, tag="rd", bufs=1, name="rD")
                nc.gpsimd.tensor_copy(rD[:], rDf[:])
                return pxv, rD

            def emit_divide(it, pc, pxv, rD):
                isl = slice(IT * it, IT * it + IT)
                for hh in range(2):
                    nsl = slice(512 * hh, 512 * hh + 512)
                    rDb = ps.tile([128, IT], F32, tag="sc", name="rDb")
                    nc.tensor.matmul(rDb[0:64, :], ones_fr[:], rD[:, nsl],
                                     start=True, stop=True)
                    rDs = wkp.tile([64, IT], BF16, tag="rds", bufs=2,
                                   name="rDs")
                    nc.vector.tensor_copy(rDs[:], rDb[0:64, :])
                    nc.vector.tensor_tensor(
                        xvT[64 * hh : 64 * hh + 64, pc, isl],
                        pxv[0:64, nsl], rDs[:], ALU.mult)

            # ---------------- emit: first units, rest via fillers -----------
            def emit_k_unit(pc, it):
                emit_qk_unit(kT[:, pc, IT * it : IT * it + IT],
                             wk, 2 + pc, pc, it)

            emit_k_unit(0, 0)
            emit_v_unit(0, 0)
            q_next = [emit_q_unit(0, 0), None]

            # late-phase inputs: DMAs emitted early (low queue priority is
            # fine - only out-projection needs them), broadcast matmuls
            # deferred into an it0 filler so they never stall the PE queue.
            wout = pp.tile([128, N_EC, D], BF16, name="wout_sb")
            bout = pp.tile([1, D], F32R, name="bout_sb")
            gamma = pp.tile([1, D], F32R, name="gamma_sb")
            beta = pp.tile([1, D], F32R, name="beta_sb")
            xres = pp.tile([128, N_EC, 512], BF16, name="xres_sb")
            gbc = pp.tile([128, D], BF16, name="gbc_sb")
            bbc = pp.tile([128, D], BF16, name="bbc_sb")
            nc.sync.dma_start(
                wout[:], wout_d.ap().rearrange("(c p) n -> p c n", p=128))
            nc.sync.dma_start(bout[:], bout_d.ap())
            nc.sync.dma_start(gamma[:], gamma_d.ap())
            nc.sync.dma_start(beta[:], beta_d.ap())
            nc.sync.dma_start(
                xres[:], xres_d.ap().rearrange("(c p) s -> p c s", p=128))

            def emit_gb_bcast():
                for src_t, dst in ((gamma, gbc), (beta, bbc)):
                    for half in range(2):
                        nsl = slice(512 * half, 512 * half + 512)
                        pbc = ps.tile([128, 512], F32, tag="sc", name="pbc")
                        nc.tensor.matmul(pbc[:], ones_bf[:], src_t[:, nsl],
                                         start=True, stop=True)
                        nc.vector.tensor_copy(dst[:, nsl], pbc[:])

            # 8-way exchange: sender block j = [256 chan, 64 tok] slice j
            # of its batch; receiver j gets batch-0 channels (senders 0-3)
            # in rows 0-1023 and batch-1 (senders 4-7) in rows 1024-2047.
            a2a_in = [dram.tile([2048, 64], BF16, name=f"a2a_in{k}")
                      for k in range(N_IT)]
            a2a_out = [dram.tile([2048, 64], BF16, name=f"a2a_out{k}")
                       for k in range(N_IT)]

            # ---------------- out-projection + layernorm chunk --------------
            def emit_outproj_stages(k, half=None):
                """Out-projection for token block k as a list of (slot, fn)
                emissions so the zk DMA latency and the matmul burst spread
                over several j-chunks instead of stalling the PE queue."""
                ki = k if half is None else 3 + half
                tw = 64 if half is None else 32
                nt = 2 * tw
                zk = wkp.tile([128, N_EC, nt], BF16, tag="zk", bufs=2,
                              name="zk")
                py = [ps.tile([nt, 512], F32, tag="sc", name=f"py{nh}")
                      for nh in range(2)]

                def s_load():
                    zsrc = a2a_out[ki].rearrange("(b e p) t -> p b e t",
                                                 p=128, b=2, e=N_EC)
                    for bh in range(2):
                        tsl = slice(tw * bh, tw * bh + tw)
                        xoff = JC * k + 64 * bh + (32 * half if half else 0)
                        nc.sync.dma_start(zk[:, :, tsl], zsrc[:, bh])
                        nc.gpsimd.tensor_tensor(
                            zk[:, :, tsl], zk[:, :, tsl],
                            xres[:, :, xoff : xoff + tw], ALU.add)

                def s_mm(nh):
                    nsl = slice(512 * nh, 512 * nh + 512)
                    for e in range(N_EC):
                        nc.tensor.matmul(py[nh][:], zk[:, e, :],
                                         wout[:, e, nsl],
                                         start=(e == 0), stop=False)
                    nc.tensor.matmul(py[nh][:], ones_bf[:, 0:nt],
                                     bout[:, nsl], start=False, stop=True)

                def s_ln():
                    emit_ln_store(k, py, half)

                return [s_load, lambda: s_mm(0), lambda: s_mm(1), s_ln]

            def emit_ln_store(k, py, half=None):
                tw = 64 if half is None else 32
                nt = 2 * tw
                bn6 = wkp.tile([128, 2, 6], F32, tag="bn6", bufs=2, name="bn6")
                nc.vector.bn_stats(bn6[0:nt, 0, :], py[0][:])
                nc.vector.bn_stats(bn6[0:nt, 1, :], py[1][:])
                bn2 = wkp.tile([128, 2], F32, tag="bn2", bufs=2, name="bn2")
                nc.vector.bn_aggr(bn2[0:nt], bn6[0:nt])
                # rstd = (var+eps)^-0.5 without Ln (keeps ScalarE on the Exp
                # table the whole kernel): Mitchell bitwise log2 on DVE ->
                # exp(-0.5 ln v) seed -> one Newton step to 3e-4 accuracy.
                vv = wkp.tile([128, 1], F32, tag="lnv", bufs=2, name="vv")
                nc.vector.tensor_scalar(vv[0:nt], bn2[0:nt, 1:2], 1e-5, None,
                                        ALU.add)
                iv = wkp.tile([128, 1], F32, tag="iv", bufs=2, name="iv")
                nc.vector.tensor_copy(iv[0:nt], vv[0:nt].bitcast(mybir.dt.int32))
                lnv = wkp.tile([128, 1], F32, tag="lnv2", bufs=2, name="lnv")
                LN2 = float(np.log(2.0))
                nc.vector.tensor_scalar(lnv[0:nt], iv[0:nt], LN2 / (1 << 23),
                                        -(127.0 - 0.0450) * LN2,
                                        ALU.mult, ALU.add)
                r0 = wkp.tile([128, 1], F32, tag="rstd0", bufs=2, name="r0")
                nc.scalar.activation(r0[0:nt], lnv[0:nt], AF.Exp, scale=-0.5)
                r2 = wkp.tile([128, 1], F32, tag="r2", bufs=2, name="r2")
                nc.vector.tensor_tensor(r2[0:nt], r0[0:nt], r0[0:nt], ALU.mult)
                nc.vector.tensor_tensor(r2[0:nt], r2[0:nt], vv[0:nt], ALU.mult)
                nc.vector.tensor_scalar(r2[0:nt], r2[0:nt], -0.5, 1.5,
                                        ALU.mult, ALU.add)
                rstd = wkp.tile([128, 1], F32, tag="rstd", bufs=2, name="rstd")
                nc.vector.tensor_tensor(rstd[0:nt], r0[0:nt], r2[0:nt], ALU.mult)
                yn = wkp.tile([128, D], BF16, tag="yn", bufs=2, name="yn")
                for nh in range(2):
                    nsl = slice(512 * nh, 512 * nh + 512)
                    t = wkp.tile([128, 512], BF16, tag="lt", bufs=2,
                                 name="lt")
                    nc.vector.tensor_scalar(t[0:nt], py[nh][:],
                                            bn2[0:nt, 0:1], rstd[0:nt],
                                            ALU.subtract, ALU.mult)
                    t2 = wkp.tile([128, 512], BF16, tag="lt2", bufs=2,
                                  name="lt2")
                    nc.vector.tensor_tensor(t2[0:nt], t[0:nt], gbc[0:nt, nsl],
                                            ALU.mult)
                    nc.vector.tensor_tensor(yn[0:nt, nsl], t2[0:nt],
                                            bbc[0:nt, nsl], ALU.add)
                if half is None:
                    nc.sync.dma_start(y_d.ap()[JC * k : JC * k + JC, :],
                                      yn[:])
                else:
                    # half h covers 32-token slices of both batch halves
                    ydst = y_d.ap()[JC * k : JC * k + JC, :].rearrange(
                        "(b t) n -> b t n", b=2)
                    nc.sync.dma_start(
                        ydst[:, 32 * half : 32 * half + 32, :],
                        yn[0:nt, :].rearrange("(b t) n -> b t n", b=2))

            # ---------------- main loop -------------------------------------
            def mkf(fn, *args):
                return lambda: fn(*args)

            pend = []  # deferred divide-epilogues: (it, pc, pxv, rD)

            def drain_divides():
                out = [mkf(emit_divide, *args) for args in pend]
                pend.clear()
                return out

            def emit_stage_a2a(it, half=None):
                ki = it if half is None else 3 + half
                tw = 64 if half is None else 32
                off = 0 if not half else 32
                a2a_dst = a2a_in[ki].rearrange("(j c p) t -> p c j t",
                                               p=128, c=2, j=8)
                src_ap = xvT[:, :, IT * it : IT * it + IT].rearrange(
                    "p c (j t) -> p c j t", j=8)
                for pc in range(2):
                    nc.sync.dma_start(a2a_dst[:, pc],
                                      src_ap[:, pc, :, off : off + tw])
                if sim:
                    # timing stand-in for TimelineSim (no collective support)
                    nc.sync.dma_start(a2a_out[ki][:], a2a_in[ki][:])
                else:
                    nc.gpsimd.collective_compute(
                        "AllToAll", ALU.bypass,
                        replica_groups=groups,
                        ins=[a2a_in[ki].opt()], outs=[a2a_out[ki].opt()])

            for it in range(N_IT):
                q0, q1 = q_next
                nq = [None, None]

                def grab0(itn):
                    nq[0] = emit_q_unit(0, itn)

                def grab1(itn):
                    nq[1] = emit_q_unit(1, itn)

                if it == 0:
                    # prime everything else under the it0 windows; pc0 only
                    # needs v(0,*) (vnat subtiles for heads 0-1), so v(1,*)
                    # rides the pc1 window
                    f0 = {
                        1: [mkf(emit_k_unit, 0, 1)],
                        3: [mkf(emit_v_unit, 0, 1)],
                        5: [mkf(emit_k_unit, 0, 2)],
                        7: [mkf(emit_v_unit, 0, 2)],
                        9: [mkf(emit_k_unit, 0, 3)],
                        11: [mkf(emit_v_unit, 0, 3)],
                        13: [mkf(emit_k_unit, 1, 0)],
                        14: [mkf(emit_v_unit, 1, 0)],
                    }
                    pxv, rD = emit_attention(0, 0, q0, f0)
                    pend.append((0, 0, pxv, rD))
                    q1 = emit_q_unit(1, 0)
                    f1 = {1: [mkf(emit_k_unit, 1, 1)],
                          4: drain_divides(),
                          3: [mkf(emit_v_unit, 1, 1)],
                          5: [mkf(emit_k_unit, 1, 2)],
                          6: [emit_gb_bcast],
                          7: [mkf(emit_v_unit, 1, 2)],
                          8: [mkf(grab0, 1)],
                          9: [mkf(emit_k_unit, 1, 3)],
                          11: [mkf(emit_v_unit, 1, 3)],
                          12: [mkf(grab1, 1)]}
                    pxv, rD = emit_attention(0, 1, q1, f1)
                    pend.append((0, 1, pxv, rD))
                else:
                    # drain prev divide, then exchange + out-project the
                    # previous token block spread over this iteration
                    f0 = {4: drain_divides()
                          + [mkf(emit_stage_a2a, it - 1)]}
                    pxv, rD = emit_attention(it, 0, q0, f0)
                    pend.append((it, 0, pxv, rD))
                    st = emit_outproj_stages(it - 1)
                    f1 = {4: drain_divides() + [st[0]],
                          6: [st[1]], 9: [st[2]], 12: [st[3]]}
                    if it < N_IT - 1:
                        f1[8] = [mkf(grab0, it + 1)]
                        f1[13] = [mkf(grab1, it + 1)]
                    pxv, rD = emit_attention(it, 1, q1, f1)
                    pend.append((it, 1, pxv, rD))
                q_next = nq
            for f in drain_divides():
                f()
            emit_stage_a2a(N_IT - 1, half=0)
            emit_stage_a2a(N_IT - 1, half=1)
            sa = emit_outproj_stages(N_IT - 1, half=0)
            sb = emit_outproj_stages(N_IT - 1, half=1)
            for s in (sa[0], sb[0], sa[1], sa[2], sb[1], sb[2], sa[3], sb[3]):
                s()

    nc.compile()
    return nc


_NC_CACHE = None


def _get_nc():
    global _NC_CACHE
    if _NC_CACHE is None:
        _NC_CACHE = _build()
    return _NC_CACHE


def _prepare_in_maps(x, w_qkv, b_qkv, w_out, b_out, ln_gamma, ln_beta):
    x = np.asarray(x, dtype=np.float32)
    w_qkv = np.asarray(w_qkv, dtype=np.float32)
    b_qkv = np.asarray(b_qkv, dtype=np.float32)
    w_out = np.ascontiguousarray(np.asarray(w_out, dtype=np.float32))
    b_out = np.asarray(b_out, dtype=np.float32)
    ln_gamma = np.asarray(ln_gamma, dtype=np.float32)
    ln_beta = np.asarray(ln_beta, dtype=np.float32)

    cosM, sinM, SpermT = _rope_consts()
    ident = np.eye(128, dtype=np.float32)
    xT = [np.ascontiguousarray(x[b].T) for b in range(B)]

    in_maps = []
    for c in range(NCORES):
        b, g = c // 4, c % 4
        col = slice(256 * g, 256 * g + 256)
        bq = b_qkv[col]
        bk = b_qkv[D:][col]
        bv = b_qkv[2 * D:][col]
        bqkv6 = np.ascontiguousarray(
            np.stack([bq[:128], bq[128:], bk[:128], bk[128:],
                      bv[:128], bv[128:]], axis=1).astype(np.float32))
        # my output tokens: per chunk k, 64 tokens of each batch at
        # 512k + 64c (c = global core id = receiver rank)
        xres = np.concatenate(
            [xT[bb][:, IT * k + 64 * c : IT * k + 64 * c + 64]
             for k in range(N_IT) for bb in range(B)], axis=1)
        m = {
            "xT": xT[b].astype(BF),
            "wq": np.ascontiguousarray(w_qkv[:, col]).astype(BF),
            "wk": np.ascontiguousarray(w_qkv[:, D:][:, col]).astype(BF),
            "wv": np.ascontiguousarray(w_qkv[:, 2 * D:][:, col]).astype(BF),
            "bqkv": bqkv6,
            "wout": w_out.astype(BF),
            "bout": b_out[None, :],
            "gamma": ln_gamma[None, :],
            "beta": ln_beta[None, :],
            "cosM": cosM.astype(BF), "sinM": sinM.astype(BF),
            "SpermT": SpermT, "ident": ident,
            "xres": np.ascontiguousarray(xres).astype(BF),
        }
        in_maps.append(m)
    return in_maps


def _assemble(results):
    out = np.zeros((B, S, D), dtype=np.float32)
    for c in range(NCORES):
        y = results[c]["y_out"]
        for k in range(N_IT):
            for bb in range(B):
                t0 = IT * k + 64 * c
                out[bb, t0 : t0 + 64, :] = \
                    y[JC * k + 64 * bb : JC * k + 64 * bb + 64]
    return out


def run(trace=False, **inputs):
    """Full run returning (output, BassKernelResults) — used by test.py for
    profiling; kernel() below is the graded entry point."""
    in_maps = _prepare_in_maps(**inputs)
    res = bass_utils.run_bass_kernel_spmd(
        _get_nc(), in_maps, core_ids=list(range(NCORES)), trace=trace)
    return _assemble(res.results), res


def kernel(**inputs):
    out, _ = run(trace=False, **inputs)
    return out


# revision 40
# speedup vs baseline: 1.1328x; 1.0003x over previous
"""Multi-head rotary attention block on 8 Trainium2 NeuronCores.

Sharding (data-parallel over batch x tensor-parallel over heads):
  core c: batch b = c//4, head group g = c%4 -> heads 4g..4g+3.
  Each core loads only its batch's x, projects q/k/v for its 4 heads,
  runs attention locally, then a 4-way AllToAll inside each batch quad
  redistributes the attention output from head-sharded to token-sharded form
  for the output projection + layernorm.

The AllToAll is chunked: after each 512-token attention tile completes, one
[1024, 128]-per-core exchange fires and that 128-token slice's output
projection + layernorm runs overlapped with the next attention tile, so the
collective+projection tail is almost fully hidden.

Matmuls run as float32r (full-rate fp32 PE mode, self-loading weights - the
bf16 path would split every matmul into LDWEIGHTS+MATMUL pairs and saturate
the PE sequencer). Softmax is exp(s/32) on ScalarE with denominators
accumulated through an extra ones-column in v, divided out via a fast DVE
reciprocal + PE broadcast. Rope is applied as raw*cos + (SpermT^T raw)*sin
with the rotation permutation as a single matmul per tile; qkv biases ride
the PSUM->SBUF copies as tensor_scalar adds instead of extra matmuls.
"""
import numpy as np
import ml_dtypes

import concourse.bass as bass
import concourse.bacc as bacc
import concourse.tile as tile
import concourse.mybir as mybir
from concourse import bass_utils

F32 = mybir.dt.float32
F32R = mybir.dt.float32r
BF16 = mybir.dt.bfloat16
AF = mybir.ActivationFunctionType
ALU = mybir.AluOpType
BF = ml_dtypes.bfloat16

NCORES = 8
B, S, D = 2, 2048, 1024
HEADS, HD = 16, 64
SCALE = 1.0 / float(np.sqrt(D))  # reference scales by full D, not head_dim
IT = 512          # i-tile width for attention / token block
N_IT = S // IT    # 4
JC = 128          # j-chunk
N_JC = S // JC    # 16
N_EC = D // 128   # 8 e-chunks


def _rope_consts():
    rot = HD // 2
    inv_freq = 1.0 / (10000.0 ** (np.arange(0, rot, 2, dtype=np.float64) / rot))
    ang = np.arange(S, dtype=np.float64)[:, None] * inv_freq[None, :]
    ang = np.repeat(ang, 2, axis=-1)  # [S, 32]
    cos, sin = np.cos(ang), np.sin(ang)
    cosM = np.ones((128, S), dtype=np.float32)
    sinM = np.zeros((128, S), dtype=np.float32)
    for base in (0, 64):
        cosM[base : base + 32, :] = cos.T.astype(np.float32)
        sinM[base : base + 32, :] = sin.T.astype(np.float32)
    Sp = np.zeros((128, 128), dtype=np.float32)
    for base in (0, 64):
        for m in range(32):
            r0 = base + m
            if m % 2 == 0:
                Sp[r0, r0 + 1] = -1.0
            else:
                Sp[r0, r0 - 1] = 1.0
    SpermT = np.ascontiguousarray(Sp.T)
    return cosM, sinM, SpermT


def _build(sim=False):
    nc = bacc.Bacc("TRN2", target_bir_lowering=False, debug=False,
                   num_devices=NCORES)

    xT_d = nc.dram_tensor("xT", [D, S], BF16, kind="ExternalInput")
    wq_d = nc.dram_tensor("wq", [D, 256], BF16, kind="ExternalInput")
    wk_d = nc.dram_tensor("wk", [D, 256], BF16, kind="ExternalInput")
    wv_d = nc.dram_tensor("wv", [D, 256], BF16, kind="ExternalInput")
    bqkv_d = nc.dram_tensor("bqkv", [128, 6], F32, kind="ExternalInput")
    wout_d = nc.dram_tensor("wout", [D, D], BF16, kind="ExternalInput")
    bout_d = nc.dram_tensor("bout", [1, D], F32R, kind="ExternalInput")
    gamma_d = nc.dram_tensor("gamma", [1, D], F32R, kind="ExternalInput")
    beta_d = nc.dram_tensor("beta", [1, D], F32R, kind="ExternalInput")
    cosM_d = nc.dram_tensor("cosM", [128, S], BF16, kind="ExternalInput")
    sinM_d = nc.dram_tensor("sinM", [128, S], BF16, kind="ExternalInput")
    spt_d = nc.dram_tensor("SpermT", [128, 128], F32R, kind="ExternalInput")
    ident_d = nc.dram_tensor("ident", [128, 128], F32R, kind="ExternalInput")
    xres_d = nc.dram_tensor("xres", [D, 512], BF16, kind="ExternalInput")
    y_d = nc.dram_tensor("y_out", [512, D], BF16, kind="ExternalOutput")

    groups = [list(range(NCORES))]

    with tile.TileContext(nc) as tc:
        with (
            tc.tile_pool(name="persist", bufs=1) as pp,
            tc.tile_pool(name="dram", bufs=1, space="DRAM") as dram,
            tc.tile_pool(name="ps", bufs=4, space="PSUM") as ps,
            tc.tile_pool(name="psacc", bufs=2, space="PSUM") as psacc,
            tc.tile_pool(name="wk", bufs=1) as wkp,
        ):
            # ---------------- input DMAs (priority ~ emission order) -------
            wq = pp.tile([128, N_EC, 256], BF16, name="wq_sb")
            wk = pp.tile([128, N_EC, 256], BF16, name="wk_sb")
            wv = pp.tile([128, N_EC, 256], BF16, name="wv_sb")
            cosM = pp.tile([128, S], BF16, name="cosM_sb")
            sinM = pp.tile([128, S], BF16, name="sinM_sb")
            spt = pp.tile([128, 128], F32R, name="spt_sb")
            ident = pp.tile([128, 128], F32R, name="ident_sb")
            bqkv = pp.tile([128, 6], F32, name="bqkv_sb")
            wup_src = pp.tile([1, 512], F32R, name="wup_src")
            nc.vector.memset(wup_src[:].bitcast(F32), 0.125)
            xt = pp.tile([128, N_EC, S], BF16, name="xt_sb")
            xt_src = xT_d.ap().rearrange("(c p) s -> p c s", p=128)

            def w_src(w_dd):
                return w_dd.ap().rearrange("(c p) m -> p c m", p=128)

            # first projection unit (k, pc0, it0) gated only by these:
            nc.sync.dma_start(wk[:], w_src(wk_d))
            nc.sync.dma_start(spt[:], spt_d.ap())
            nc.sync.dma_start(bqkv[:], bqkv_d.ap())
            nc.sync.dma_start(cosM[:], cosM_d.ap())
            nc.sync.dma_start(sinM[:], sinM_d.ap())
            nc.sync.dma_start(xt[:, 0:4, 0:IT], xt_src[:, 0:4, 0:IT])
            nc.sync.dma_start(xt[:, 4:8, 0:IT], xt_src[:, 4:8, 0:IT])
            nc.sync.dma_start(wq[:], w_src(wq_d))
            nc.sync.dma_start(wv[:], w_src(wv_d))
            nc.sync.dma_start(ident[:], ident_d.ap())
            for it in range(1, N_IT):
                isl = slice(IT * it, IT * it + IT)
                nc.sync.dma_start(xt[:, :, isl], xt_src[:, :, isl])

            ones_bf = pp.tile([1, 128], F32R, name="ones_bf")
            nc.vector.memset(ones_bf[:].bitcast(F32), 1.0)
            ones_fr = pp.tile([1, 64], F32R, name="ones_fr")
            nc.vector.memset(ones_fr[:].bitcast(F32), 1.0)
            # PE warmup: dep-free matmuls fill the DMA-bound prolog so the
            # tensor engine reaches full clock before real work arrives
            wup = ps.tile([128, 512], F32, tag="sc", name="wup")
            for _ in range(18):
                nc.tensor.matmul(wup[:], ones_bf[:], wup_src[:],
                                 start=True, stop=True)
            eps_sb = pp.tile([128, 1], F32, name="eps_sb")
            nc.vector.memset(eps_sb[:], 1e-5)

            kT = pp.tile([128, 2, S], F32R, name="kT_sb")
            vnat = [pp.tile([128, 260], F32R, name=f"vnat_{j}")
                    for j in range(N_JC)]
            for j in range(N_JC):
                nc.vector.memset(vnat[j][:, 64::65].bitcast(F32), 1.0)
            xvT = pp.tile([128, 2, S], BF16, name="xvT_sb")

            # ---------------- projection helpers ---------------------------
            # Units are split into phase1 (PSUM accumulation + bias copy) and
            # phase2 (rope / transposes). One phase2 stays pending so the next
            # unit's matmuls fill the PE while DVE finishes the previous
            # unit's bias add - the in-order PE queue never waits on DVE.
            unit_pipe = []

            def pump_units(f2=None):
                while unit_pipe:
                    unit_pipe.pop(0)()
                if f2 is not None:
                    unit_pipe.append(f2)

            def emit_qk_unit(dst_ap, w_sb, bcol, pc, it):
                # dst_ap: [128, 512] destination (bf16 kT slice or f32r q tile)
                isl = slice(IT * it, IT * it + IT)
                praw = ps.tile([128, IT], F32, tag="sc", name="praw")
                for e in range(N_EC):
                    nc.tensor.matmul(praw[:],
                                     w_sb[:, e, 128 * pc : 128 * pc + 128],
                                     xt[:, e, isl],
                                     start=(e == 0), stop=(e == N_EC - 1))
                raw = wkp.tile([128, IT], F32R, tag="raw", bufs=3, name="raw")
                nc.vector.tensor_scalar(raw[:], praw[:],
                                        bqkv[:, bcol : bcol + 1], None, ALU.add)

                def phase2(dst_ap=dst_ap):
                    prot = ps.tile([128, IT], F32, tag="sc", name="prot")
                    nc.tensor.matmul(prot[:], spt[:], raw[:],
                                     start=True, stop=True)
                    t1 = wkp.tile([128, IT], BF16, tag="t1", bufs=2,
                                  name="t1")
                    nc.vector.tensor_tensor(t1[:], prot[:], sinM[:, isl],
                                            ALU.mult)
                    t2 = wkp.tile([128, IT], BF16, tag="t2", bufs=2,
                                  name="t2")
                    nc.gpsimd.tensor_tensor(t2[:], raw[:].bitcast(F32),
                                            cosM[:, isl], ALU.mult)
                    nc.vector.tensor_tensor(dst_ap, t1[:], t2[:],
                                            ALU.add)

                pump_units(phase2)

            def emit_q_unit(pc, it):
                q_t = wkp.tile([128, IT], F32R, tag="qt", bufs=3, name="q_t")
                emit_qk_unit(q_t[:], wq, pc, pc, it)
                return q_t

            def emit_v_unit(pc, it):
                isl = slice(IT * it, IT * it + IT)
                pvt = ps.tile([128, IT], F32, tag="sc", name="pvt")
                for e in range(N_EC):
                    nc.tensor.matmul(pvt[:],
                                     wv[:, e, 128 * pc : 128 * pc + 128],
                                     xt[:, e, isl],
                                     start=(e == 0), stop=(e == N_EC - 1))
                vt = wkp.tile([128, IT], F32R, tag="vt", bufs=2, name="vt")
                nc.vector.tensor_scalar(vt[:], pvt[:],
                                        bqkv[:, 4 + pc : 5 + pc], None, ALU.add)

                def phase2():
                    for jj in range(IT // JC):
                        jcc = it * (IT // JC) + jj
                        ptr = ps.tile([128, 128], F32R, tag="sc", name="ptr")
                        nc.tensor.transpose(
                            ptr[:], vt[:, JC * jj : JC * jj + JC], ident[:])
                        for hh in range(2):
                            h = 2 * pc + hh
                            nc.vector.tensor_copy(
                                vnat[jcc][:, 65 * h : 65 * h + 64],
                                ptr[:, 64 * hh : 64 * hh + 64].bitcast(F32))

                pump_units(phase2)

            # ---------------- attention helper ------------------------------
            def emit_attention(it, pc, q_t, fillers=None):
                pump_units()
                isl = slice(IT * it, IT * it + IT)
                pxv = psacc.tile([128, 1024], F32, tag="acc", name="pxv")
                for jc in range(N_JC):
                    pump_units()  # pending phase2 lands 1 j-chunk after its
                    if fillers and jc in fillers:  # phase1 - always in time
                        for f in fillers[jc]:
                            f()
                    jsl = slice(JC * jc, JC * jc + JC)
                    for hh in range(2):
                        h = 2 * pc + hh
                        hsl = slice(64 * hh, 64 * hh + 64)
                        psc = ps.tile([128, IT], F32, tag="sc", name="psc")
                        nc.tensor.matmul(psc[:], kT[hsl, pc, jsl],
                                         q_t[hsl, :],
                                         start=True, stop=True)
                        pt = wkp.tile([128, IT], F32R, tag="pt", bufs=6,
                                      name="pt")
                        nc.scalar.activation(pt[:], psc[:], AF.Exp, scale=SCALE)
                        nc.tensor.matmul(
                            pxv[0:65, 512 * hh : 512 * hh + 512],
                            vnat[jc][:, 65 * h : 65 * h + 65],
                            pt[:], start=(jc == 0), stop=(jc == N_JC - 1))
                # softmax denominator reciprocal (broadcast + multiply are
                # deferred into the next tile-group's fillers so their
                # dep-stalls never block the in-order PE queue)
                rDf = wkp.tile([1, 1024], F32, tag="rdf", bufs=1, name="rDf")
                nc.vector.reciprocal_approx_fast(rDf[:], pxv[64:65, :])
                rD = wkp.tile([1, 1024], F32R, tag="rd", bufs=1, name="rD")
                nc.gpsimd.tensor_copy(rD[:], rDf[:])
                return pxv, rD

            def emit_divide(it, pc, pxv, rD):
                isl = slice(IT * it, IT * it + IT)
                for hh in range(2):
                    nsl = slice(512 * hh, 512 * hh + 512)
                    rDb = ps.tile([128, IT], F32, tag="sc", name="rDb")
                    nc.tensor.matmul(rDb[0:64, :], ones_fr[:], rD[:, nsl],
                                     start=True, stop=True)
                    rDs = wkp.tile([64, IT], BF16, tag="rds", bufs=4,
                                   name="rDs")
                    nc.vector.tensor_copy(rDs[:], rDb[0:64, :])
                    nc.vector.tensor_tensor(
                        xvT[64 * hh : 64 * hh + 64, pc, isl],
                        pxv[0:64, nsl], rDs[:], ALU.mult)

            # ---------------- emit: first units, rest via fillers -----------
            def emit_k_unit(pc, it):
                emit_qk_unit(kT[:, pc, IT * it : IT * it + IT],
                             wk, 2 + pc, pc, it)

            emit_k_unit(0, 0)
            emit_v_unit(0, 0)
            q_next = [emit_q_unit(0, 0), None]

            # late-phase inputs: DMAs emitted early (low queue priority is
            # fine - only out-projection needs them), broadcast matmuls
            # deferred into an it0 filler so they never stall the PE queue.
            wout = pp.tile([128, N_EC, D], BF16, name="wout_sb")
            bout = pp.tile([1, D], F32R, name="bout_sb")
            gamma = pp.tile([1, D], F32R, name="gamma_sb")
            beta = pp.tile([1, D], F32R, name="beta_sb")
            xres = pp.tile([128, N_EC, 512], BF16, name="xres_sb")
            gbc = pp.tile([128, D], BF16, name="gbc_sb")
            bbc = pp.tile([128, D], BF16, name="bbc_sb")
            nc.sync.dma_start(
                wout[:], wout_d.ap().rearrange("(c p) n -> p c n", p=128))
            nc.sync.dma_start(bout[:], bout_d.ap())
            nc.sync.dma_start(gamma[:], gamma_d.ap())
            nc.sync.dma_start(beta[:], beta_d.ap())
            nc.sync.dma_start(
                xres[:], xres_d.ap().rearrange("(c p) s -> p c s", p=128))

            def emit_gb_bcast():
                for src_t, dst in ((gamma, gbc), (beta, bbc)):
                    for half in range(2):
                        nsl = slice(512 * half, 512 * half + 512)
                        pbc = ps.tile([128, 512], F32, tag="sc", name="pbc")
                        nc.tensor.matmul(pbc[:], ones_bf[:], src_t[:, nsl],
                                         start=True, stop=True)
                        nc.vector.tensor_copy(dst[:, nsl], pbc[:])

            # 8-way exchange: sender block j = [256 chan, 64 tok] slice j
            # of its batch; receiver j gets batch-0 channels (senders 0-3)
            # in rows 0-1023 and batch-1 (senders 4-7) in rows 1024-2047.
            a2a_in = [dram.tile([2048, 64], BF16, name=f"a2a_in{k}")
                      for k in range(N_IT)]
            a2a_out = [dram.tile([2048, 64], BF16, name=f"a2a_out{k}")
                       for k in range(N_IT)]

            # ---------------- out-projection + layernorm chunk --------------
            def emit_outproj_stages(k, half=None):
                """Out-projection for token block k as a list of (slot, fn)
                emissions so the zk DMA latency and the matmul burst spread
                over several j-chunks instead of stalling the PE queue."""
                ki = k if half is None else 3 + half
                tw = 64 if half is None else 32
                nt = 2 * tw
                zk = wkp.tile([128, N_EC, nt], BF16, tag="zk", bufs=2,
                              name="zk")
                py = [ps.tile([nt, 512], F32, tag="sc", name=f"py{nh}")
                      for nh in range(2)]

                def s_load():
                    zsrc = a2a_out[ki].rearrange("(b e p) t -> p b e t",
                                                 p=128, b=2, e=N_EC)
                    for bh in range(2):
                        tsl = slice(tw * bh, tw * bh + tw)
                        xoff = JC * k + 64 * bh + (32 * half if half else 0)
                        nc.sync.dma_start(zk[:, :, tsl], zsrc[:, bh])
                        nc.gpsimd.tensor_tensor(
                            zk[:, :, tsl], zk[:, :, tsl],
                            xres[:, :, xoff : xoff + tw], ALU.add)

                def s_mm(nh):
                    nsl = slice(512 * nh, 512 * nh + 512)
                    for e in range(N_EC):
                        nc.tensor.matmul(py[nh][:], zk[:, e, :],
                                         wout[:, e, nsl],
                                         start=(e == 0), stop=False)
                    nc.tensor.matmul(py[nh][:], ones_bf[:, 0:nt],
                                     bout[:, nsl], start=False, stop=True)

                def s_ln():
                    emit_ln_store(k, py, half)

                return [s_load, lambda: s_mm(0), lambda: s_mm(1), s_ln]

            def emit_ln_store(k, py, half=None):
                tw = 64 if half is None else 32
                nt = 2 * tw
                bn6 = wkp.tile([128, 2, 6], F32, tag="bn6", bufs=2, name="bn6")
                nc.vector.bn_stats(bn6[0:nt, 0, :], py[0][:])
                nc.vector.bn_stats(bn6[0:nt, 1, :], py[1][:])
                bn2 = wkp.tile([128, 2], F32, tag="bn2", bufs=2, name="bn2")
                nc.vector.bn_aggr(bn2[0:nt], bn6[0:nt])
                # rstd = (var+eps)^-0.5 without Ln (keeps ScalarE on the Exp
                # table the whole kernel): Mitchell bitwise log2 on DVE ->
                # exp(-0.5 ln v) seed -> one Newton step to 3e-4 accuracy.
                vv = wkp.tile([128, 1], F32, tag="lnv", bufs=2, name="vv")
                nc.vector.tensor_scalar(vv[0:nt], bn2[0:nt, 1:2], 1e-5, None,
                                        ALU.add)
                iv = wkp.tile([128, 1], F32, tag="iv", bufs=2, name="iv")
                nc.vector.tensor_copy(iv[0:nt], vv[0:nt].bitcast(mybir.dt.int32))
                lnv = wkp.tile([128, 1], F32, tag="lnv2", bufs=2, name="lnv")
                LN2 = float(np.log(2.0))
                nc.vector.tensor_scalar(lnv[0:nt], iv[0:nt], LN2 / (1 << 23),
                                        -(127.0 - 0.0450) * LN2,
                                        ALU.mult, ALU.add)
                r0 = wkp.tile([128, 1], F32, tag="rstd0", bufs=2, name="r0")
                nc.scalar.activation(r0[0:nt], lnv[0:nt], AF.Exp, scale=-0.5)
                r2 = wkp.tile([128, 1], F32, tag="r2", bufs=2, name="r2")
                nc.vector.tensor_tensor(r2[0:nt], r0[0:nt], r0[0:nt], ALU.mult)
                nc.vector.tensor_tensor(r2[0:nt], r2[0:nt], vv[0:nt], ALU.mult)
                nc.vector.tensor_scalar(r2[0:nt], r2[0:nt], -0.5, 1.5,
                                        ALU.mult, ALU.add)
                rstd = wkp.tile([128, 1], F32, tag="rstd", bufs=2, name="rstd")
                nc.vector.tensor_tensor(rstd[0:nt], r0[0:nt], r2[0:nt], ALU.mult)
                yn = wkp.tile([128, D], BF16, tag="yn", bufs=2, name="yn")
                for nh in range(2):
                    nsl = slice(512 * nh, 512 * nh + 512)
                    t = wkp.tile([128, 512], BF16, tag="lt", bufs=2,
                                 name="lt")
                    nc.vector.tensor_scalar(t[0:nt], py[nh][:],
                                            bn2[0:nt, 0:1], rstd[0:nt],
                                            ALU.subtract, ALU.mult)
                    t2 = wkp.tile([128, 512], BF16, tag="lt2", bufs=2,
                                  name="lt2")
                    nc.vector.tensor_tensor(t2[0:nt], t[0:nt], gbc[0:nt, nsl],
                                            ALU.mult)
                    nc.vector.tensor_tensor(yn[0:nt, nsl], t2[0:nt],
                                            bbc[0:nt, nsl], ALU.add)
                if half is None:
                    nc.sync.dma_start(y_d.ap()[JC * k : JC * k + JC, :],
                                      yn[:])
                else:
                    # half h covers 32-token slices of both batch halves
                    ydst = y_d.ap()[JC * k : JC * k + JC, :].rearrange(
                        "(b t) n -> b t n", b=2)
                    nc.sync.dma_start(
                        ydst[:, 32 * half : 32 * half + 32, :],
                        yn[0:nt, :].rearrange("(b t) n -> b t n", b=2))

            # ---------------- main loop -------------------------------------
            def mkf(fn, *args):
                return lambda: fn(*args)

            pend = []  # deferred divide-epilogues: (it, pc, pxv, rD)

            def drain_divides():
                out = [mkf(emit_divide, *args) for args in pend]
                pend.clear()
                return out

            def emit_stage_a2a(it, half=None):
                ki = it if half is None else 3 + half
                tw = 64 if half is None else 32
                off = 0 if not half else 32
                a2a_dst = a2a_in[ki].rearrange("(j c p) t -> p c j t",
                                               p=128, c=2, j=8)
                src_ap = xvT[:, :, IT * it : IT * it + IT].rearrange(
                    "p c (j t) -> p c j t", j=8)
                for pc in range(2):
                    nc.sync.dma_start(a2a_dst[:, pc],
                                      src_ap[:, pc, :, off : off + tw])
                if sim:
                    # timing stand-in for TimelineSim (no collective support)
                    nc.sync.dma_start(a2a_out[ki][:], a2a_in[ki][:])
                else:
                    nc.gpsimd.collective_compute(
                        "AllToAll", ALU.bypass,
                        replica_groups=groups,
                        ins=[a2a_in[ki].opt()], outs=[a2a_out[ki].opt()])

            for it in range(N_IT):
                q0, q1 = q_next
                nq = [None, None]

                def grab0(itn):
                    nq[0] = emit_q_unit(0, itn)

                def grab1(itn):
                    nq[1] = emit_q_unit(1, itn)

                if it == 0:
                    # prime everything else under the it0 windows; pc0 only
                    # needs v(0,*) (vnat subtiles for heads 0-1), so v(1,*)
                    # rides the pc1 window
                    f0 = {
                        1: [mkf(emit_k_unit, 0, 1)],
                        3: [mkf(emit_v_unit, 0, 1)],
                        5: [mkf(emit_k_unit, 0, 2)],
                        7: [mkf(emit_v_unit, 0, 2)],
                        9: [mkf(emit_k_unit, 0, 3)],
                        11: [mkf(emit_v_unit, 0, 3)],
                        13: [mkf(emit_k_unit, 1, 0)],
                        14: [mkf(emit_v_unit, 1, 0)],
                    }
                    pxv, rD = emit_attention(0, 0, q0, f0)
                    pend.append((0, 0, pxv, rD))
                    q1 = emit_q_unit(1, 0)
                    f1 = {1: [mkf(emit_k_unit, 1, 1)],
                          4: drain_divides(),
                          3: [mkf(emit_v_unit, 1, 1)],
                          5: [mkf(emit_k_unit, 1, 2)],
                          6: [emit_gb_bcast],
                          7: [mkf(emit_v_unit, 1, 2)],
                          8: [mkf(grab0, 1)],
                          9: [mkf(emit_k_unit, 1, 3)],
                          11: [mkf(emit_v_unit, 1, 3)],
                          12: [mkf(grab1, 1)]}
                    pxv, rD = emit_attention(0, 1, q1, f1)
                    pend.append((0, 1, pxv, rD))
                else:
                    # drain prev divide, then exchange + out-project the
                    # previous token block spread over this iteration
                    f0 = {4: drain_divides()
                          + [mkf(emit_stage_a2a, it - 1)]}
                    pxv, rD = emit_attention(it, 0, q0, f0)
                    pend.append((it, 0, pxv, rD))
                    st = emit_outproj_stages(it - 1)
                    f1 = {4: drain_divides() + [st[0]],
                          6: [st[1]], 9: [st[2]], 12: [st[3]]}
                    if it < N_IT - 1:
                        f1[8] = [mkf(grab0, it + 1)]
                        f1[13] = [mkf(grab1, it + 1)]
                    pxv, rD = emit_attention(it, 1, q1, f1)
                    pend.append((it, 1, pxv, rD))
                q_next = nq
            for f in drain_divides():
                f()
            emit_stage_a2a(N_IT - 1, half=0)
            emit_stage_a2a(N_IT - 1, half=1)
            sa = emit_outproj_stages(N_IT - 1, half=0)
            sb = emit_outproj_stages(N_IT - 1, half=1)
            for s in (sa[0], sb[0], sa[1], sa[2], sb[1], sb[2], sa[3], sb[3]):
                s()

    nc.compile()
    return nc


_NC_CACHE = None


def _get_nc():
    global _NC_CACHE
    if _NC_CACHE is None:
        _NC_CACHE = _build()
    return _NC_CACHE


def _prepare_in_maps(x, w_qkv, b_qkv, w_out, b_out, ln_gamma, ln_beta):
    x = np.asarray(x, dtype=np.float32)
    w_qkv = np.asarray(w_qkv, dtype=np.float32)
    b_qkv = np.asarray(b_qkv, dtype=np.float32)
    w_out = np.ascontiguousarray(np.asarray(w_out, dtype=np.float32))
    b_out = np.asarray(b_out, dtype=np.float32)
    ln_gamma = np.asarray(ln_gamma, dtype=np.float32)
    ln_beta = np.asarray(ln_beta, dtype=np.float32)

    cosM, sinM, SpermT = _rope_consts()
    ident = np.eye(128, dtype=np.float32)
    xT = [np.ascontiguousarray(x[b].T) for b in range(B)]

    in_maps = []
    for c in range(NCORES):
        b, g = c // 4, c % 4
        col = slice(256 * g, 256 * g + 256)
        bq = b_qkv[col]
        bk = b_qkv[D:][col]
        bv = b_qkv[2 * D:][col]
        bqkv6 = np.ascontiguousarray(
            np.stack([bq[:128], bq[128:], bk[:128], bk[128:],
                      bv[:128], bv[128:]], axis=1).astype(np.float32))
        # my output tokens: per chunk k, 64 tokens of each batch at
        # 512k + 64c (c = global core id = receiver rank)
        xres = np.concatenate(
            [xT[bb][:, IT * k + 64 * c : IT * k + 64 * c + 64]
             for k in range(N_IT) for bb in range(B)], axis=1)
        m = {
            "xT": xT[b].astype(BF),
            "wq": np.ascontiguousarray(w_qkv[:, col]).astype(BF),
            "wk": np.ascontiguousarray(w_qkv[:, D:][:, col]).astype(BF),
            "wv": np.ascontiguousarray(w_qkv[:, 2 * D:][:, col]).astype(BF),
            "bqkv": bqkv6,
            "wout": w_out.astype(BF),
            "bout": b_out[None, :],
            "gamma": ln_gamma[None, :],
            "beta": ln_beta[None, :],
            "cosM": cosM.astype(BF), "sinM": sinM.astype(BF),
            "SpermT": SpermT, "ident": ident,
            "xres": np.ascontiguousarray(xres).astype(BF),
        }
        in_maps.append(m)
    return in_maps


def _assemble(results):
    out = np.zeros((B, S, D), dtype=np.float32)
    for c in range(NCORES):
        y = results[c]["y_out"]
        for k in range(N_IT):
            for bb in range(B):
                t0 = IT * k + 64 * c
                out[bb, t0 : t0 + 64, :] = \
                    y[JC * k + 64 * bb : JC * k + 64 * bb + 64]
    return out


def run(trace=False, **inputs):
    """Full run returning (output, BassKernelResults) — used by test.py for
    profiling; kernel() below is the graded entry point."""
    in_maps = _prepare_in_maps(**inputs)
    res = bass_utils.run_bass_kernel_spmd(
        _get_nc(), in_maps, core_ids=list(range(NCORES)), trace=trace)
    return _assemble(res.results), res


def kernel(**inputs):
    out, _ = run(trace=False, **inputs)
    return out


# revision 42
# speedup vs baseline: 1.1336x; 1.0008x over previous
"""Multi-head rotary attention block on 8 Trainium2 NeuronCores.

Sharding (data-parallel over batch x tensor-parallel over heads):
  core c: batch b = c//4, head group g = c%4 -> heads 4g..4g+3.
  Each core loads only its batch's x, projects q/k/v for its 4 heads,
  runs attention locally, then a 4-way AllToAll inside each batch quad
  redistributes the attention output from head-sharded to token-sharded form
  for the output projection + layernorm.

The AllToAll is chunked: after each 512-token attention tile completes, one
[1024, 128]-per-core exchange fires and that 128-token slice's output
projection + layernorm runs overlapped with the next attention tile, so the
collective+projection tail is almost fully hidden.

Matmuls run as float32r (full-rate fp32 PE mode, self-loading weights - the
bf16 path would split every matmul into LDWEIGHTS+MATMUL pairs and saturate
the PE sequencer). Softmax is exp(s/32) on ScalarE with denominators
accumulated through an extra ones-column in v, divided out via a fast DVE
reciprocal + PE broadcast. Rope is applied as raw*cos + (SpermT^T raw)*sin
with the rotation permutation as a single matmul per tile; qkv biases ride
the PSUM->SBUF copies as tensor_scalar adds instead of extra matmuls.
"""
import numpy as np
import ml_dtypes

import concourse.bass as bass
import concourse.bacc as bacc
import concourse.tile as tile
import concourse.mybir as mybir
from concourse import bass_utils

F32 = mybir.dt.float32
F32R = mybir.dt.float32r
BF16 = mybir.dt.bfloat16
AF = mybir.ActivationFunctionType
ALU = mybir.AluOpType
BF = ml_dtypes.bfloat16

NCORES = 8
B, S, D = 2, 2048, 1024
HEADS, HD = 16, 64
SCALE = 1.0 / float(np.sqrt(D))  # reference scales by full D, not head_dim
IT = 512          # i-tile width for attention / token block
N_IT = S // IT    # 4
JC = 128          # j-chunk
N_JC = S // JC    # 16
N_EC = D // 128   # 8 e-chunks


def _rope_consts():
    rot = HD // 2
    inv_freq = 1.0 / (10000.0 ** (np.arange(0, rot, 2, dtype=np.float64) / rot))
    ang = np.arange(S, dtype=np.float64)[:, None] * inv_freq[None, :]
    ang = np.repeat(ang, 2, axis=-1)  # [S, 32]
    cos, sin = np.cos(ang), np.sin(ang)
    cosM = np.ones((128, S), dtype=np.float32)
    sinM = np.zeros((128, S), dtype=np.float32)
    for base in (0, 64):
        cosM[base : base + 32, :] = cos.T.astype(np.float32)
        sinM[base : base + 32, :] = sin.T.astype(np.float32)
    Sp = np.zeros((128, 128), dtype=np.float32)
    for base in (0, 64):
        for m in range(32):
            r0 = base + m
            if m % 2 == 0:
                Sp[r0, r0 + 1] = -1.0
            else:
                Sp[r0, r0 - 1] = 1.0
    SpermT = np.ascontiguousarray(Sp.T)
    return cosM, sinM, SpermT


def _build(sim=False):
    nc = bacc.Bacc("TRN2", target_bir_lowering=False, debug=False,
                   num_devices=NCORES)

    xT_d = nc.dram_tensor("xT", [D, S], BF16, kind="ExternalInput")
    wq_d = nc.dram_tensor("wq", [D, 256], BF16, kind="ExternalInput")
    wk_d = nc.dram_tensor("wk", [D, 256], BF16, kind="ExternalInput")
    wv_d = nc.dram_tensor("wv", [D, 256], BF16, kind="ExternalInput")
    bqkv_d = nc.dram_tensor("bqkv", [128, 6], F32, kind="ExternalInput")
    wout_d = nc.dram_tensor("wout", [D, D], BF16, kind="ExternalInput")
    bout_d = nc.dram_tensor("bout", [1, D], F32R, kind="ExternalInput")
    gamma_d = nc.dram_tensor("gamma", [1, D], F32R, kind="ExternalInput")
    beta_d = nc.dram_tensor("beta", [1, D], F32R, kind="ExternalInput")
    cosM_d = nc.dram_tensor("cosM", [128, S], BF16, kind="ExternalInput")
    sinM_d = nc.dram_tensor("sinM", [128, S], BF16, kind="ExternalInput")
    spt_d = nc.dram_tensor("SpermT", [128, 128], F32R, kind="ExternalInput")
    ident_d = nc.dram_tensor("ident", [128, 128], F32R, kind="ExternalInput")
    xres_d = nc.dram_tensor("xres", [D, 512], BF16, kind="ExternalInput")
    y_d = nc.dram_tensor("y_out", [512, D], BF16, kind="ExternalOutput")

    groups = [list(range(NCORES))]

    with tile.TileContext(nc) as tc:
        with (
            tc.tile_pool(name="persist", bufs=1) as pp,
            tc.tile_pool(name="dram", bufs=1, space="DRAM") as dram,
            tc.tile_pool(name="ps", bufs=4, space="PSUM") as ps,
            tc.tile_pool(name="psacc", bufs=2, space="PSUM") as psacc,
            tc.tile_pool(name="wk", bufs=1) as wkp,
        ):
            # ---------------- input DMAs (priority ~ emission order) -------
            wq = pp.tile([128, N_EC, 256], BF16, name="wq_sb")
            wk = pp.tile([128, N_EC, 256], BF16, name="wk_sb")
            wv = pp.tile([128, N_EC, 256], BF16, name="wv_sb")
            cosM = pp.tile([128, S], BF16, name="cosM_sb")
            sinM = pp.tile([128, S], BF16, name="sinM_sb")
            spt = pp.tile([128, 128], F32R, name="spt_sb")
            ident = pp.tile([128, 128], F32R, name="ident_sb")
            bqkv = pp.tile([128, 6], F32, name="bqkv_sb")
            wup_src = pp.tile([1, 512], F32R, name="wup_src")
            nc.vector.memset(wup_src[:].bitcast(F32), 0.125)
            xt = pp.tile([128, N_EC, S], BF16, name="xt_sb")
            xt_src = xT_d.ap().rearrange("(c p) s -> p c s", p=128)

            def w_src(w_dd):
                return w_dd.ap().rearrange("(c p) m -> p c m", p=128)

            # first projection unit (k, pc0, it0) gated only by these:
            nc.sync.dma_start(wk[:], w_src(wk_d))
            nc.sync.dma_start(spt[:], spt_d.ap())
            nc.sync.dma_start(bqkv[:], bqkv_d.ap())
            nc.sync.dma_start(cosM[:], cosM_d.ap())
            nc.sync.dma_start(sinM[:], sinM_d.ap())
            nc.sync.dma_start(xt[:, 0:4, 0:IT], xt_src[:, 0:4, 0:IT])
            nc.sync.dma_start(xt[:, 4:8, 0:IT], xt_src[:, 4:8, 0:IT])
            nc.sync.dma_start(wq[:], w_src(wq_d))
            nc.sync.dma_start(wv[:], w_src(wv_d))
            nc.sync.dma_start(ident[:], ident_d.ap())
            for it in range(1, N_IT):
                isl = slice(IT * it, IT * it + IT)
                nc.sync.dma_start(xt[:, :, isl], xt_src[:, :, isl])

            ones_bf = pp.tile([1, 128], F32R, name="ones_bf")
            nc.vector.memset(ones_bf[:].bitcast(F32), 1.0)
            ones_fr = pp.tile([1, 64], F32R, name="ones_fr")
            nc.vector.memset(ones_fr[:].bitcast(F32), 1.0)
            # PE warmup: dep-free matmuls fill the DMA-bound prolog so the
            # tensor engine reaches full clock before real work arrives
            wup = ps.tile([128, 512], F32, tag="sc", name="wup")
            for _ in range(18):
                nc.tensor.matmul(wup[:], ones_bf[:], wup_src[:],
                                 start=True, stop=True)
            eps_sb = pp.tile([128, 1], F32, name="eps_sb")
            nc.vector.memset(eps_sb[:], 1e-5)

            kT = pp.tile([128, 2, S], F32R, name="kT_sb")
            vnat = [pp.tile([128, 260], F32R, name=f"vnat_{j}")
                    for j in range(N_JC)]
            for j in range(N_JC):
                nc.vector.memset(vnat[j][:, 64::65].bitcast(F32), 1.0)
            xvT = pp.tile([128, 2, S], BF16, name="xvT_sb")

            # ---------------- projection helpers ---------------------------
            # Units are split into phase1 (PSUM accumulation + bias copy) and
            # phase2 (rope / transposes). One phase2 stays pending so the next
            # unit's matmuls fill the PE while DVE finishes the previous
            # unit's bias add - the in-order PE queue never waits on DVE.
            unit_pipe = []

            def pump_units(f2=None):
                while unit_pipe:
                    unit_pipe.pop(0)()
                if f2 is not None:
                    unit_pipe.append(f2)

            def emit_qk_unit(dst_ap, w_sb, bcol, pc, it):
                # dst_ap: [128, 512] destination (bf16 kT slice or f32r q tile)
                isl = slice(IT * it, IT * it + IT)
                praw = ps.tile([128, IT], F32, tag="sc", name="praw")
                for e in range(N_EC):
                    nc.tensor.matmul(praw[:],
                                     w_sb[:, e, 128 * pc : 128 * pc + 128],
                                     xt[:, e, isl],
                                     start=(e == 0), stop=(e == N_EC - 1))
                raw = wkp.tile([128, IT], F32R, tag="raw", bufs=3, name="raw")
                nc.vector.tensor_scalar(raw[:], praw[:],
                                        bqkv[:, bcol : bcol + 1], None, ALU.add)

                def phase2(dst_ap=dst_ap):
                    prot = ps.tile([128, IT], F32, tag="sc", name="prot")
                    nc.tensor.matmul(prot[:], spt[:], raw[:],
                                     start=True, stop=True)
                    t1 = wkp.tile([128, IT], BF16, tag="t1", bufs=2,
                                  name="t1")
                    nc.vector.tensor_tensor(t1[:], prot[:], sinM[:, isl],
                                            ALU.mult)
                    t2 = wkp.tile([128, IT], BF16, tag="t2", bufs=2,
                                  name="t2")
                    nc.gpsimd.tensor_tensor(t2[:], raw[:].bitcast(F32),
                                            cosM[:, isl], ALU.mult)
                    nc.vector.tensor_tensor(dst_ap, t1[:], t2[:],
                                            ALU.add)

                pump_units(phase2)

            def emit_q_unit(pc, it):
                q_t = wkp.tile([128, IT], F32R, tag="qt", bufs=3, name="q_t")
                emit_qk_unit(q_t[:], wq, pc, pc, it)
                return q_t

            def emit_v_unit(pc, it):
                isl = slice(IT * it, IT * it + IT)
                pvt = ps.tile([128, IT], F32, tag="sc", name="pvt")
                for e in range(N_EC):
                    nc.tensor.matmul(pvt[:],
                                     wv[:, e, 128 * pc : 128 * pc + 128],
                                     xt[:, e, isl],
                                     start=(e == 0), stop=(e == N_EC - 1))
                vt = wkp.tile([128, IT], F32R, tag="vt", bufs=2, name="vt")
                nc.vector.tensor_scalar(vt[:], pvt[:],
                                        bqkv[:, 4 + pc : 5 + pc], None, ALU.add)

                def phase2():
                    for jj in range(IT // JC):
                        jcc = it * (IT // JC) + jj
                        ptr = ps.tile([128, 128], F32R, tag="sc", name="ptr")
                        nc.tensor.transpose(
                            ptr[:], vt[:, JC * jj : JC * jj + JC], ident[:])
                        for hh in range(2):
                            h = 2 * pc + hh
                            nc.vector.tensor_copy(
                                vnat[jcc][:, 65 * h : 65 * h + 64],
                                ptr[:, 64 * hh : 64 * hh + 64].bitcast(F32))

                pump_units(phase2)

            # ---------------- attention helper ------------------------------
            def emit_attention(it, pc, q_t, fillers=None):
                pump_units()
                isl = slice(IT * it, IT * it + IT)
                pxv = psacc.tile([128, 1024], F32, tag="acc", name="pxv")
                for jc in range(N_JC):
                    pump_units()  # pending phase2 lands 1 j-chunk after its
                    if fillers and jc in fillers:  # phase1 - always in time
                        for f in fillers[jc]:
                            f()
                    jsl = slice(JC * jc, JC * jc + JC)
                    for hh in range(2):
                        h = 2 * pc + hh
                        hsl = slice(64 * hh, 64 * hh + 64)
                        psc = ps.tile([128, IT], F32, tag="sc", name="psc")
                        nc.tensor.matmul(psc[:], kT[hsl, pc, jsl],
                                         q_t[hsl, :],
                                         start=True, stop=True)
                        pt = wkp.tile([128, IT], F32R, tag="pt", bufs=6,
                                      name="pt")
                        nc.scalar.activation(pt[:], psc[:], AF.Exp, scale=SCALE)
                        nc.tensor.matmul(
                            pxv[0:65, 512 * hh : 512 * hh + 512],
                            vnat[jc][:, 65 * h : 65 * h + 65],
                            pt[:], start=(jc == 0), stop=(jc == N_JC - 1))
                # softmax denominator reciprocal (broadcast + multiply are
                # deferred into the next tile-group's fillers so their
                # dep-stalls never block the in-order PE queue)
                rDf = wkp.tile([1, 1024], F32, tag="rdf", bufs=1, name="rDf")
                nc.vector.reciprocal_approx_fast(rDf[:], pxv[64:65, :])
                rD = wkp.tile([1, 1024], F32R, tag="rd", bufs=1, name="rD")
                nc.gpsimd.tensor_copy(rD[:], rDf[:])
                return pxv, rD

            def emit_divide(it, pc, pxv, rD):
                isl = slice(IT * it, IT * it + IT)
                for hh in range(2):
                    nsl = slice(512 * hh, 512 * hh + 512)
                    rDb = ps.tile([128, IT], F32, tag="sc", name="rDb")
                    nc.tensor.matmul(rDb[0:64, :], ones_fr[:], rD[:, nsl],
                                     start=True, stop=True)
                    rDs = wkp.tile([64, IT], BF16, tag="rds", bufs=4,
                                   name="rDs")
                    nc.vector.tensor_copy(rDs[:], rDb[0:64, :])
                    nc.vector.tensor_tensor(
                        xvT[64 * hh : 64 * hh + 64, pc, isl],
                        pxv[0:64, nsl], rDs[:], ALU.mult)

            # ---------------- emit: first units, rest via fillers -----------
            def emit_k_unit(pc, it):
                emit_qk_unit(kT[:, pc, IT * it : IT * it + IT],
                             wk, 2 + pc, pc, it)

            emit_k_unit(0, 0)
            emit_v_unit(0, 0)
            q_next = [emit_q_unit(0, 0), None]

            # late-phase inputs: DMAs emitted early (low queue priority is
            # fine - only out-projection needs them), broadcast matmuls
            # deferred into an it0 filler so they never stall the PE queue.
            wout = pp.tile([128, N_EC, D], BF16, name="wout_sb")
            bout = pp.tile([1, D], F32R, name="bout_sb")
            gamma = pp.tile([1, D], F32R, name="gamma_sb")
            beta = pp.tile([1, D], F32R, name="beta_sb")
            xres = pp.tile([128, N_EC, 512], BF16, name="xres_sb")
            gbc = pp.tile([128, D], BF16, name="gbc_sb")
            bbc = pp.tile([128, D], BF16, name="bbc_sb")
            nc.sync.dma_start(
                wout[:], wout_d.ap().rearrange("(c p) n -> p c n", p=128))
            nc.sync.dma_start(bout[:], bout_d.ap())
            nc.sync.dma_start(gamma[:], gamma_d.ap())
            nc.sync.dma_start(beta[:], beta_d.ap())
            nc.sync.dma_start(
                xres[:], xres_d.ap().rearrange("(c p) s -> p c s", p=128))

            def emit_gb_bcast():
                for src_t, dst in ((gamma, gbc), (beta, bbc)):
                    for half in range(2):
                        nsl = slice(512 * half, 512 * half + 512)
                        pbc = ps.tile([128, 512], F32, tag="sc", name="pbc")
                        nc.tensor.matmul(pbc[:], ones_bf[:], src_t[:, nsl],
                                         start=True, stop=True)
                        nc.vector.tensor_copy(dst[:, nsl], pbc[:])

            # 8-way exchange: sender block j = [256 chan, 64 tok] slice j
            # of its batch; receiver j gets batch-0 channels (senders 0-3)
            # in rows 0-1023 and batch-1 (senders 4-7) in rows 1024-2047.
            a2a_in = [dram.tile([2048, 64], BF16, name=f"a2a_in{k}")
                      for k in range(N_IT)]
            a2a_out = [dram.tile([2048, 64], BF16, name=f"a2a_out{k}")
                       for k in range(N_IT)]

            # ---------------- out-projection + layernorm chunk --------------
            def emit_outproj_stages(k, half=None):
                """Out-projection for token block k as a list of (slot, fn)
                emissions so the zk DMA latency and the matmul burst spread
                over several j-chunks instead of stalling the PE queue."""
                ki = k if half is None else 3 + half
                tw = 64 if half is None else 32
                nt = 2 * tw
                zk = wkp.tile([128, N_EC, nt], BF16, tag="zk", bufs=2,
                              name="zk")
                py = [ps.tile([nt, 512], F32, tag="sc", name=f"py{nh}")
                      for nh in range(2)]

                def s_load():
                    zsrc = a2a_out[ki].rearrange("(b e p) t -> p b e t",
                                                 p=128, b=2, e=N_EC)
                    for bh in range(2):
                        tsl = slice(tw * bh, tw * bh + tw)
                        xoff = JC * k + 64 * bh + (32 * half if half else 0)
                        nc.sync.dma_start(zk[:, :, tsl], zsrc[:, bh])
                        nc.gpsimd.tensor_tensor(
                            zk[:, :, tsl], zk[:, :, tsl],
                            xres[:, :, xoff : xoff + tw], ALU.add)

                def s_mm(nh):
                    nsl = slice(512 * nh, 512 * nh + 512)
                    for e in range(N_EC):
                        nc.tensor.matmul(py[nh][:], zk[:, e, :],
                                         wout[:, e, nsl],
                                         start=(e == 0), stop=False)
                    nc.tensor.matmul(py[nh][:], ones_bf[:, 0:nt],
                                     bout[:, nsl], start=False, stop=True)

                def s_ln():
                    emit_ln_store(k, py, half)

                return [s_load, lambda: s_mm(0), lambda: s_mm(1), s_ln]

            def emit_ln_store(k, py, half=None):
                tw = 64 if half is None else 32
                nt = 2 * tw
                bn6 = wkp.tile([128, 2, 6], F32, tag="bn6", bufs=2, name="bn6")
                nc.vector.bn_stats(bn6[0:nt, 0, :], py[0][:])
                nc.vector.bn_stats(bn6[0:nt, 1, :], py[1][:])
                bn2 = wkp.tile([128, 2], F32, tag="bn2", bufs=2, name="bn2")
                nc.vector.bn_aggr(bn2[0:nt], bn6[0:nt])
                # rstd = (var+eps)^-0.5 without Ln (keeps ScalarE on the Exp
                # table the whole kernel): Mitchell bitwise log2 on DVE ->
                # exp(-0.5 ln v) seed -> one Newton step to 3e-4 accuracy.
                vv = wkp.tile([128, 1], F32, tag="lnv", bufs=2, name="vv")
                nc.vector.tensor_scalar(vv[0:nt], bn2[0:nt, 1:2], 1e-5, None,
                                        ALU.add)
                iv = wkp.tile([128, 1], F32, tag="iv", bufs=2, name="iv")
                nc.vector.tensor_copy(iv[0:nt], vv[0:nt].bitcast(mybir.dt.int32))
                lnv = wkp.tile([128, 1], F32, tag="lnv2", bufs=2, name="lnv")
                LN2 = float(np.log(2.0))
                nc.vector.tensor_scalar(lnv[0:nt], iv[0:nt], LN2 / (1 << 23),
                                        -(127.0 - 0.0450) * LN2,
                                        ALU.mult, ALU.add)
                r0 = wkp.tile([128, 1], F32, tag="rstd0", bufs=2, name="r0")
                nc.scalar.activation(r0[0:nt], lnv[0:nt], AF.Exp, scale=-0.5)
                r2 = wkp.tile([128, 1], F32, tag="r2", bufs=2, name="r2")
                nc.vector.tensor_tensor(r2[0:nt], r0[0:nt], r0[0:nt], ALU.mult)
                nc.vector.tensor_tensor(r2[0:nt], r2[0:nt], vv[0:nt], ALU.mult)
                nc.vector.tensor_scalar(r2[0:nt], r2[0:nt], -0.5, 1.5,
                                        ALU.mult, ALU.add)
                rstd = wkp.tile([128, 1], F32, tag="rstd", bufs=2, name="rstd")
                nc.vector.tensor_tensor(rstd[0:nt], r0[0:nt], r2[0:nt], ALU.mult)
                yn = wkp.tile([128, D], BF16, tag="yn", bufs=2, name="yn")
                for nh in range(2):
                    nsl = slice(512 * nh, 512 * nh + 512)
                    t = wkp.tile([128, 512], BF16, tag="lt", bufs=2,
                                 name="lt")
                    nc.vector.tensor_scalar(t[0:nt], py[nh][:],
                                            bn2[0:nt, 0:1], rstd[0:nt],
                                            ALU.subtract, ALU.mult)
                    t2 = wkp.tile([128, 512], BF16, tag="lt2", bufs=2,
                                  name="lt2")
                    nc.vector.tensor_tensor(t2[0:nt], t[0:nt], gbc[0:nt, nsl],
                                            ALU.mult)
                    nc.vector.tensor_tensor(yn[0:nt, nsl], t2[0:nt],
                                            bbc[0:nt, nsl], ALU.add)
                if half is None:
                    nc.sync.dma_start(y_d.ap()[JC * k : JC * k + JC, :],
                                      yn[:])
                else:
                    # half h covers 32-token slices of both batch halves
                    ydst = y_d.ap()[JC * k : JC * k + JC, :].rearrange(
                        "(b t) n -> b t n", b=2)
                    nc.sync.dma_start(
                        ydst[:, 32 * half : 32 * half + 32, :],
                        yn[0:nt, :].rearrange("(b t) n -> b t n", b=2))

            # ---------------- main loop -------------------------------------
            def mkf(fn, *args):
                return lambda: fn(*args)

            pend = []  # deferred divide-epilogues: (it, pc, pxv, rD)

            def drain_divides():
                out = [mkf(emit_divide, *args) for args in pend]
                pend.clear()
                return out

            def emit_stage_a2a(it, half=None):
                ki = it if half is None else 3 + half
                tw = 64 if half is None else 32
                off = 0 if not half else 32
                a2a_dst = a2a_in[ki].rearrange("(j c p) t -> p c j t",
                                               p=128, c=2, j=8)
                src_ap = xvT[:, :, IT * it : IT * it + IT].rearrange(
                    "p c (j t) -> p c j t", j=8)
                for pc in range(2):
                    nc.sync.dma_start(a2a_dst[:, pc],
                                      src_ap[:, pc, :, off : off + tw])
                if sim:
                    # timing stand-in for TimelineSim (no collective support)
                    nc.sync.dma_start(a2a_out[ki][:], a2a_in[ki][:])
                else:
                    nc.gpsimd.collective_compute(
                        "AllToAll", ALU.bypass,
                        replica_groups=groups,
                        ins=[a2a_in[ki].opt()], outs=[a2a_out[ki].opt()])

            for it in range(N_IT):
                q0, q1 = q_next
                nq = [None, None]

                def grab0(itn):
                    nq[0] = emit_q_unit(0, itn)

                def grab1(itn):
                    nq[1] = emit_q_unit(1, itn)

                if it == 0:
                    # prime everything else under the it0 windows; pc0 only
                    # needs v(0,*) (vnat subtiles for heads 0-1), so v(1,*)
                    # rides the pc1 window
                    f0 = {
                        1: [mkf(emit_k_unit, 0, 1)],
                        3: [mkf(emit_v_unit, 0, 1)],
                        5: [mkf(emit_k_unit, 0, 2)],
                        7: [mkf(emit_v_unit, 0, 2)],
                        9: [mkf(emit_k_unit, 0, 3)],
                        11: [mkf(emit_v_unit, 0, 3)],
                        13: [mkf(emit_k_unit, 1, 0)],
                        14: [mkf(emit_v_unit, 1, 0)],
                    }
                    pxv, rD = emit_attention(0, 0, q0, f0)
                    pend.append((0, 0, pxv, rD))
                    q1 = emit_q_unit(1, 0)
                    f1 = {1: [mkf(emit_k_unit, 1, 1)],
                          4: drain_divides(),
                          3: [mkf(emit_v_unit, 1, 1)],
                          5: [mkf(emit_k_unit, 1, 2)],
                          6: [emit_gb_bcast],
                          7: [mkf(emit_v_unit, 1, 2)],
                          8: [mkf(grab0, 1)],
                          9: [mkf(emit_k_unit, 1, 3)],
                          11: [mkf(emit_v_unit, 1, 3)],
                          12: [mkf(grab1, 1)]}
                    pxv, rD = emit_attention(0, 1, q1, f1)
                    pend.append((0, 1, pxv, rD))
                else:
                    # drain prev divide, then exchange + out-project the
                    # previous token block spread over this iteration
                    f0 = {4: drain_divides()
                          + [mkf(emit_stage_a2a, it - 1)]}
                    pxv, rD = emit_attention(it, 0, q0, f0)
                    pend.append((it, 0, pxv, rD))
                    st = emit_outproj_stages(it - 1)
                    f1 = {4: drain_divides() + [st[0]],
                          6: [st[1]], 9: [st[2]], 12: [st[3]]}
                    if it < N_IT - 1:
                        f1[8] = [mkf(grab0, it + 1)]
                        f1[13] = [mkf(grab1, it + 1)]
                    pxv, rD = emit_attention(it, 1, q1, f1)
                    pend.append((it, 1, pxv, rD))
                q_next = nq
            for f in drain_divides():
                f()
            emit_stage_a2a(N_IT - 1, half=0)
            emit_stage_a2a(N_IT - 1, half=1)
            sa = emit_outproj_stages(N_IT - 1, half=0)
            sb = emit_outproj_stages(N_IT - 1, half=1)
            for s in (sa[0], sb[0], sa[1], sa[2], sb[1], sb[2], sa[3], sb[3]):
                s()

    nc.compile()
    return nc


_NC_CACHE = None


def _get_nc():
    global _NC_CACHE
    if _NC_CACHE is None:
        _NC_CACHE = _build()
    return _NC_CACHE


def _prepare_in_maps(x, w_qkv, b_qkv, w_out, b_out, ln_gamma, ln_beta):
    x = np.asarray(x, dtype=np.float32)
    w_qkv = np.asarray(w_qkv, dtype=np.float32)
    b_qkv = np.asarray(b_qkv, dtype=np.float32)
    w_out = np.ascontiguousarray(np.asarray(w_out, dtype=np.float32))
    b_out = np.asarray(b_out, dtype=np.float32)
    ln_gamma = np.asarray(ln_gamma, dtype=np.float32)
    ln_beta = np.asarray(ln_beta, dtype=np.float32)

    cosM, sinM, SpermT = _rope_consts()
    ident = np.eye(128, dtype=np.float32)
    xT = [np.ascontiguousarray(x[b].T) for b in range(B)]

    in_maps = []
    for c in range(NCORES):
        b, g = c // 4, c % 4
        col = slice(256 * g, 256 * g + 256)
        bq = b_qkv[col]
        bk = b_qkv[D:][col]
        bv = b_qkv[2 * D:][col]
        bqkv6 = np.ascontiguousarray(
            np.stack([bq[:128], bq[128:], bk[:128], bk[128:],
                      bv[:128], bv[128:]], axis=1).astype(np.float32))
        # my output tokens: per chunk k, 64 tokens of each batch at
        # 512k + 64c (c = global core id = receiver rank)
        xres = np.concatenate(
            [xT[bb][:, IT * k + 64 * c : IT * k + 64 * c + 64]
             for k in range(N_IT) for bb in range(B)], axis=1)
        m = {
            "xT": xT[b].astype(BF),
            "wq": np.ascontiguousarray(w_qkv[:, col]).astype(BF),
            "wk": np.ascontiguousarray(w_qkv[:, D:][:, col]).astype(BF),
            "wv": np.ascontiguousarray(w_qkv[:, 2 * D:][:, col]).astype(BF),
            "bqkv": bqkv6,
            "wout": w_out.astype(BF),
            "bout": b_out[None, :],
            "gamma": ln_gamma[None, :],
            "beta": ln_beta[None, :],
            "cosM": cosM.astype(BF), "sinM": sinM.astype(BF),
            "SpermT": SpermT, "ident": ident,
            "xres": np.ascontiguousarray(xres).astype(BF),
        }
        in_maps.append(m)
    return in_maps


def _assemble(results):
    out = np.zeros((B, S, D), dtype=np.float32)
    for c in range(NCORES):
        y = results[c]["y_out"]
        for k in range(N_IT):
            for bb in range(B):
                t0 = IT * k + 64 * c
                out[bb, t0 : t0 + 64, :] = \
                    y[JC * k + 64 * bb : JC * k + 64 * bb + 64]
    return out


def run(trace=False, **inputs):
    """Full run returning (output, BassKernelResults) — used by test.py for
    profiling; kernel() below is the graded entry point."""
    in_maps = _prepare_in_maps(**inputs)
    res = bass_utils.run_bass_kernel_spmd(
        _get_nc(), in_maps, core_ids=list(range(NCORES)), trace=trace)
    return _assemble(res.results), res


def kernel(**inputs):
    out, _ = run(trace=False, **inputs)
    return out


# revision 44
# speedup vs baseline: 1.1524x; 1.0166x over previous
"""Multi-head rotary attention block on 8 Trainium2 NeuronCores.

Sharding (data-parallel over batch x tensor-parallel over heads):
  core c: batch b = c//4, head group g = c%4 -> heads 4g..4g+3.
  Each core loads only its batch's x, projects q/k/v for its 4 heads,
  runs attention locally, then a 4-way AllToAll inside each batch quad
  redistributes the attention output from head-sharded to token-sharded form
  for the output projection + layernorm.

The AllToAll is chunked: after each 512-token attention tile completes, one
[1024, 128]-per-core exchange fires and that 128-token slice's output
projection + layernorm runs overlapped with the next attention tile, so the
collective+projection tail is almost fully hidden.

Matmuls run as float32r (full-rate fp32 PE mode, self-loading weights - the
bf16 path would split every matmul into LDWEIGHTS+MATMUL pairs and saturate
the PE sequencer). Softmax is exp(s/32) on ScalarE with denominators
accumulated through an extra ones-column in v, divided out via a fast DVE
reciprocal + PE broadcast. Rope is applied as raw*cos + (SpermT^T raw)*sin
with the rotation permutation as a single matmul per tile; qkv biases ride
the PSUM->SBUF copies as tensor_scalar adds instead of extra matmuls.
"""
import numpy as np
import ml_dtypes

import concourse.bass as bass
import concourse.bacc as bacc
import concourse.tile as tile
import concourse.mybir as mybir
from concourse import bass_utils

F32 = mybir.dt.float32
F32R = mybir.dt.float32r
BF16 = mybir.dt.bfloat16
AF = mybir.ActivationFunctionType
ALU = mybir.AluOpType
BF = ml_dtypes.bfloat16

NCORES = 8
B, S, D = 2, 2048, 1024
HEADS, HD = 16, 64
SCALE = 1.0 / float(np.sqrt(D))  # reference scales by full D, not head_dim
IT = 512          # i-tile width for attention / token block
N_IT = S // IT    # 4
JC = 128          # j-chunk
N_JC = S // JC    # 16
N_EC = D // 128   # 8 e-chunks


def _rope_consts():
    rot = HD // 2
    inv_freq = 1.0 / (10000.0 ** (np.arange(0, rot, 2, dtype=np.float64) / rot))
    ang = np.arange(S, dtype=np.float64)[:, None] * inv_freq[None, :]
    ang = np.repeat(ang, 2, axis=-1)  # [S, 32]
    cos, sin = np.cos(ang), np.sin(ang)
    cosM = np.ones((128, S), dtype=np.float32)
    sinM = np.zeros((128, S), dtype=np.float32)
    for base in (0, 64):
        cosM[base : base + 32, :] = cos.T.astype(np.float32)
        sinM[base : base + 32, :] = sin.T.astype(np.float32)
    Sp = np.zeros((128, 128), dtype=np.float32)
    for base in (0, 64):
        for m in range(32):
            r0 = base + m
            if m % 2 == 0:
                Sp[r0, r0 + 1] = -1.0
            else:
                Sp[r0, r0 - 1] = 1.0
    SpermT = np.ascontiguousarray(Sp.T)
    return cosM, sinM, SpermT


def _build(sim=False):
    nc = bacc.Bacc("TRN2", target_bir_lowering=False, debug=False,
                   num_devices=NCORES)

    xT_d = nc.dram_tensor("xT", [D, S], BF16, kind="ExternalInput")
    wq_d = nc.dram_tensor("wq", [D, 256], BF16, kind="ExternalInput")
    wk_d = nc.dram_tensor("wk", [D, 256], BF16, kind="ExternalInput")
    wv_d = nc.dram_tensor("wv", [D, 256], BF16, kind="ExternalInput")
    bqkv_d = nc.dram_tensor("bqkv", [128, 6], F32, kind="ExternalInput")
    wout_d = nc.dram_tensor("wout", [D, D], BF16, kind="ExternalInput")
    bout_d = nc.dram_tensor("bout", [1, D], F32R, kind="ExternalInput")
    gamma_d = nc.dram_tensor("gamma", [1, D], F32R, kind="ExternalInput")
    beta_d = nc.dram_tensor("beta", [1, D], F32R, kind="ExternalInput")
    cosM_d = nc.dram_tensor("cosM", [128, S], BF16, kind="ExternalInput")
    sinM_d = nc.dram_tensor("sinM", [128, S], BF16, kind="ExternalInput")
    spt_d = nc.dram_tensor("SpermT", [128, 128], F32R, kind="ExternalInput")
    ident_d = nc.dram_tensor("ident", [128, 128], F32R, kind="ExternalInput")
    xres_d = nc.dram_tensor("xres", [D, 512], BF16, kind="ExternalInput")
    y_d = nc.dram_tensor("y_out", [512, D], BF16, kind="ExternalOutput")

    groups = [list(range(NCORES))]

    with tile.TileContext(nc) as tc:
        with (
            tc.tile_pool(name="persist", bufs=1) as pp,
            tc.tile_pool(name="dram", bufs=1, space="DRAM") as dram,
            tc.tile_pool(name="ps", bufs=4, space="PSUM") as ps,
            tc.tile_pool(name="psacc", bufs=2, space="PSUM") as psacc,
            tc.tile_pool(name="wk", bufs=1) as wkp,
        ):
            # ---------------- input DMAs (priority ~ emission order) -------
            wq = pp.tile([128, N_EC, 256], BF16, name="wq_sb")
            wk = pp.tile([128, N_EC, 256], BF16, name="wk_sb")
            wv = pp.tile([128, N_EC, 256], BF16, name="wv_sb")
            cosM = pp.tile([128, S], BF16, name="cosM_sb")
            sinM = pp.tile([128, S], BF16, name="sinM_sb")
            spt = pp.tile([128, 128], F32R, name="spt_sb")
            ident = pp.tile([128, 128], F32R, name="ident_sb")
            bqkv = pp.tile([128, 6], F32, name="bqkv_sb")
            wup_src = pp.tile([1, 512], F32R, name="wup_src")
            nc.vector.memset(wup_src[:].bitcast(F32), 0.125)
            xt = pp.tile([128, N_EC, S], BF16, name="xt_sb")
            xt_src = xT_d.ap().rearrange("(c p) s -> p c s", p=128)

            def w_src(w_dd):
                return w_dd.ap().rearrange("(c p) m -> p c m", p=128)

            # first projection unit (k, pc0, it0) gated only by these:
            nc.sync.dma_start(wk[:], w_src(wk_d))
            nc.sync.dma_start(spt[:], spt_d.ap())
            nc.sync.dma_start(bqkv[:], bqkv_d.ap())
            nc.sync.dma_start(cosM[:], cosM_d.ap())
            nc.sync.dma_start(sinM[:], sinM_d.ap())
            nc.sync.dma_start(xt[:, 0:4, 0:IT], xt_src[:, 0:4, 0:IT])
            nc.sync.dma_start(xt[:, 4:8, 0:IT], xt_src[:, 4:8, 0:IT])
            nc.sync.dma_start(wq[:], w_src(wq_d))
            nc.sync.dma_start(wv[:], w_src(wv_d))
            nc.sync.dma_start(ident[:], ident_d.ap())
            for it in range(1, N_IT):
                isl = slice(IT * it, IT * it + IT)
                nc.sync.dma_start(xt[:, :, isl], xt_src[:, :, isl])

            ones_bf = pp.tile([1, 128], F32R, name="ones_bf")
            nc.vector.memset(ones_bf[:].bitcast(F32), 1.0)
            ones_fr = pp.tile([1, 64], F32R, name="ones_fr")
            nc.vector.memset(ones_fr[:].bitcast(F32), 1.0)
            # PE warmup: dep-free matmuls fill the DMA-bound prolog so the
            # tensor engine reaches full clock before real work arrives
            wup = ps.tile([128, 512], F32, tag="sc", name="wup")
            for _ in range(18):
                nc.tensor.matmul(wup[:], ones_bf[:], wup_src[:],
                                 start=True, stop=True)
            eps_sb = pp.tile([128, 1], F32, name="eps_sb")
            nc.vector.memset(eps_sb[:], 1e-5)

            kT = pp.tile([128, 2, S], F32R, name="kT_sb")
            vnat = [pp.tile([128, 260], F32R, name=f"vnat_{j}")
                    for j in range(N_JC)]
            for j in range(N_JC):
                nc.vector.memset(vnat[j][:, 64::65].bitcast(F32), 1.0)
            xvT = pp.tile([128, 2, S], BF16, name="xvT_sb")

            # ---------------- projection helpers ---------------------------
            # Units are split into phase1 (PSUM accumulation + bias copy) and
            # phase2 (rope / transposes). One phase2 stays pending so the next
            # unit's matmuls fill the PE while DVE finishes the previous
            # unit's bias add - the in-order PE queue never waits on DVE.
            unit_pipe = []

            def pump_units(f2=None):
                while unit_pipe:
                    unit_pipe.pop(0)()
                if f2 is not None:
                    unit_pipe.append(f2)

            def emit_qk_unit(dst_ap, w_sb, bcol, pc, it):
                # dst_ap: [128, 512] destination (bf16 kT slice or f32r q tile)
                isl = slice(IT * it, IT * it + IT)
                praw = ps.tile([128, IT], F32, tag="sc", name="praw")
                for e in range(N_EC):
                    nc.tensor.matmul(praw[:],
                                     w_sb[:, e, 128 * pc : 128 * pc + 128],
                                     xt[:, e, isl],
                                     start=(e == 0), stop=(e == N_EC - 1))
                raw = wkp.tile([128, IT], F32R, tag="raw", bufs=3, name="raw")
                nc.vector.tensor_scalar(raw[:], praw[:],
                                        bqkv[:, bcol : bcol + 1], None, ALU.add)

                def phase2(dst_ap=dst_ap):
                    prot = ps.tile([128, IT], F32, tag="sc", name="prot")
                    nc.tensor.matmul(prot[:], spt[:], raw[:],
                                     start=True, stop=True)
                    t1 = wkp.tile([128, IT], BF16, tag="t1", bufs=2,
                                  name="t1")
                    nc.vector.tensor_tensor(t1[:], prot[:], sinM[:, isl],
                                            ALU.mult)
                    t2 = wkp.tile([128, IT], BF16, tag="t2", bufs=2,
                                  name="t2")
                    nc.gpsimd.tensor_tensor(t2[:], raw[:].bitcast(F32),
                                            cosM[:, isl], ALU.mult)
                    nc.vector.tensor_tensor(dst_ap, t1[:], t2[:],
                                            ALU.add)

                pump_units(phase2)

            def emit_q_unit(pc, it):
                q_t = wkp.tile([128, IT], F32R, tag="qt", bufs=3, name="q_t")
                emit_qk_unit(q_t[:], wq, pc, pc, it)
                return q_t

            def emit_v_unit(pc, it):
                isl = slice(IT * it, IT * it + IT)
                pvt = ps.tile([128, IT], F32, tag="sc", name="pvt")
                for e in range(N_EC):
                    nc.tensor.matmul(pvt[:],
                                     wv[:, e, 128 * pc : 128 * pc + 128],
                                     xt[:, e, isl],
                                     start=(e == 0), stop=(e == N_EC - 1))
                vt = wkp.tile([128, IT], F32R, tag="vt", bufs=2, name="vt")
                nc.vector.tensor_scalar(vt[:], pvt[:],
                                        bqkv[:, 4 + pc : 5 + pc], None, ALU.add)

                def phase2():
                    for jj in range(IT // JC):
                        jcc = it * (IT // JC) + jj
                        ptr = ps.tile([128, 128], F32R, tag="sc", name="ptr")
                        nc.tensor.transpose(
                            ptr[:], vt[:, JC * jj : JC * jj + JC], ident[:])
                        for hh in range(2):
                            h = 2 * pc + hh
                            nc.vector.tensor_copy(
                                vnat[jcc][:, 65 * h : 65 * h + 64],
                                ptr[:, 64 * hh : 64 * hh + 64].bitcast(F32))

                pump_units(phase2)

            # ---------------- attention helper ------------------------------
            def emit_attention(it, pc, q_t, fillers=None):
                pump_units()
                isl = slice(IT * it, IT * it + IT)
                pxv = psacc.tile([128, 1024], F32, tag="acc", name="pxv")
                for jc in range(N_JC):
                    pump_units()  # pending phase2 lands 1 j-chunk after its
                    if fillers and jc in fillers:  # phase1 - always in time
                        for f in fillers[jc]:
                            f()
                    jsl = slice(JC * jc, JC * jc + JC)
                    for hh in range(2):
                        h = 2 * pc + hh
                        hsl = slice(64 * hh, 64 * hh + 64)
                        psc = ps.tile([128, IT], F32, tag="sc", name="psc")
                        nc.tensor.matmul(psc[:], kT[hsl, pc, jsl],
                                         q_t[hsl, :],
                                         start=True, stop=True)
                        pt = wkp.tile([128, IT], F32R, tag="pt", bufs=6,
                                      name="pt")
                        nc.scalar.activation(pt[:], psc[:], AF.Exp, scale=SCALE)
                        nc.tensor.matmul(
                            pxv[0:65, 512 * hh : 512 * hh + 512],
                            vnat[jc][:, 65 * h : 65 * h + 65],
                            pt[:], start=(jc == 0), stop=(jc == N_JC - 1))
                # softmax denominator reciprocal (broadcast + multiply are
                # deferred into the next tile-group's fillers so their
                # dep-stalls never block the in-order PE queue)
                rDf = wkp.tile([1, 1024], F32, tag="rdf", bufs=1, name="rDf")
                nc.vector.reciprocal_approx_fast(rDf[:], pxv[64:65, :])
                rD = wkp.tile([1, 1024], F32R, tag="rd", bufs=1, name="rD")
                nc.gpsimd.tensor_copy(rD[:], rDf[:])
                return pxv, rD

            def emit_divide(it, pc, pxv, rD):
                isl = slice(IT * it, IT * it + IT)
                for hh in range(2):
                    nsl = slice(512 * hh, 512 * hh + 512)
                    rDb = ps.tile([128, IT], F32, tag="sc", name="rDb")
                    nc.tensor.matmul(rDb[0:64, :], ones_fr[:], rD[:, nsl],
                                     start=True, stop=True)
                    rDs = wkp.tile([64, IT], BF16, tag="rds", bufs=4,
                                   name="rDs")
                    nc.vector.tensor_copy(rDs[:], rDb[0:64, :])
                    nc.vector.tensor_tensor(
                        xvT[64 * hh : 64 * hh + 64, pc, isl],
                        pxv[0:64, nsl], rDs[:], ALU.mult)

            # ---------------- emit: first units, rest via fillers -----------
            def emit_k_unit(pc, it):
                emit_qk_unit(kT[:, pc, IT * it : IT * it + IT],
                             wk, 2 + pc, pc, it)

            emit_k_unit(0, 0)
            emit_v_unit(0, 0)
            q_next = [emit_q_unit(0, 0), None]

            # late-phase inputs: DMAs emitted early (low queue priority is
            # fine - only out-projection needs them), broadcast matmuls
            # deferred into an it0 filler so they never stall the PE queue.
            wout = pp.tile([128, N_EC, D], BF16, name="wout_sb")
            bout = pp.tile([1, D], F32R, name="bout_sb")
            gamma = pp.tile([1, D], F32R, name="gamma_sb")
            beta = pp.tile([1, D], F32R, name="beta_sb")
            xres = pp.tile([128, N_EC, 512], BF16, name="xres_sb")
            gbc = pp.tile([128, D], BF16, name="gbc_sb")
            bbc = pp.tile([128, D], BF16, name="bbc_sb")
            nc.sync.dma_start(
                wout[:], wout_d.ap().rearrange("(c p) n -> p c n", p=128))
            nc.sync.dma_start(bout[:], bout_d.ap())
            nc.sync.dma_start(gamma[:], gamma_d.ap())
            nc.sync.dma_start(beta[:], beta_d.ap())
            nc.sync.dma_start(
                xres[:], xres_d.ap().rearrange("(c p) s -> p c s", p=128))

            def emit_gb_bcast():
                for src_t, dst in ((gamma, gbc), (beta, bbc)):
                    for half in range(2):
                        nsl = slice(512 * half, 512 * half + 512)
                        pbc = ps.tile([128, 512], F32, tag="sc", name="pbc")
                        nc.tensor.matmul(pbc[:], ones_bf[:], src_t[:, nsl],
                                         start=True, stop=True)
                        nc.vector.tensor_copy(dst[:, nsl], pbc[:])

            # 8-way exchange: sender block j = [256 chan, 64 tok] slice j
            # of its batch; receiver j gets batch-0 channels (senders 0-3)
            # in rows 0-1023 and batch-1 (senders 4-7) in rows 1024-2047.
            a2a_in = [dram.tile([2048, 64], BF16, name=f"a2a_in{k}")
                      for k in range(N_IT)]
            a2a_out = [dram.tile([2048, 64], BF16, name=f"a2a_out{k}")
                       for k in range(N_IT)]

            # ---------------- out-projection + layernorm chunk --------------
            def emit_outproj_stages(k, half=None):
                """Out-projection for token block k as a list of (slot, fn)
                emissions so the zk DMA latency and the matmul burst spread
                over several j-chunks instead of stalling the PE queue."""
                ki = k if half is None else 3 + half
                tw = 64 if half is None else 32
                nt = 2 * tw
                zk = wkp.tile([128, N_EC, nt], BF16, tag="zk", bufs=2,
                              name="zk")
                py = [ps.tile([nt, 512], F32, tag="sc", name=f"py{nh}")
                      for nh in range(2)]

                def s_load():
                    zsrc = a2a_out[ki].rearrange("(b e p) t -> p b e t",
                                                 p=128, b=2, e=N_EC)
                    for bh in range(2):
                        tsl = slice(tw * bh, tw * bh + tw)
                        xoff = JC * k + 64 * bh + (32 * half if half else 0)
                        nc.sync.dma_start(zk[:, :, tsl], zsrc[:, bh])
                        nc.gpsimd.tensor_tensor(
                            zk[:, :, tsl], zk[:, :, tsl],
                            xres[:, :, xoff : xoff + tw], ALU.add)

                def s_mm(nh):
                    nsl = slice(512 * nh, 512 * nh + 512)
                    for e in range(N_EC):
                        nc.tensor.matmul(py[nh][:], zk[:, e, :],
                                         wout[:, e, nsl],
                                         start=(e == 0), stop=False)
                    nc.tensor.matmul(py[nh][:], ones_bf[:, 0:nt],
                                     bout[:, nsl], start=False, stop=True)

                def s_ln():
                    emit_ln_store(k, py, half)

                return [s_load, lambda: s_mm(0), lambda: s_mm(1), s_ln]

            def emit_ln_store(k, py, half=None):
                tw = 64 if half is None else 32
                nt = 2 * tw
                bn6 = wkp.tile([128, 2, 6], F32, tag="bn6", bufs=2, name="bn6")
                nc.vector.bn_stats(bn6[0:nt, 0, :], py[0][:])
                nc.vector.bn_stats(bn6[0:nt, 1, :], py[1][:])
                bn2 = wkp.tile([128, 2], F32, tag="bn2", bufs=2, name="bn2")
                nc.vector.bn_aggr(bn2[0:nt], bn6[0:nt])
                # rstd = (var+eps)^-0.5 without Ln (keeps ScalarE on the Exp
                # table the whole kernel): Mitchell bitwise log2 on DVE ->
                # exp(-0.5 ln v) seed -> one Newton step to 3e-4 accuracy.
                vv = wkp.tile([128, 1], F32, tag="lnv", bufs=2, name="vv")
                nc.vector.tensor_scalar(vv[0:nt], bn2[0:nt, 1:2], 1e-5, None,
                                        ALU.add)
                iv = wkp.tile([128, 1], F32, tag="iv", bufs=2, name="iv")
                nc.vector.tensor_copy(iv[0:nt], vv[0:nt].bitcast(mybir.dt.int32))
                lnv = wkp.tile([128, 1], F32, tag="lnv2", bufs=2, name="lnv")
                LN2 = float(np.log(2.0))
                nc.vector.tensor_scalar(lnv[0:nt], iv[0:nt], LN2 / (1 << 23),
                                        -(127.0 - 0.0450) * LN2,
                                        ALU.mult, ALU.add)
                r0 = wkp.tile([128, 1], F32, tag="rstd0", bufs=2, name="r0")
                nc.scalar.activation(r0[0:nt], lnv[0:nt], AF.Exp, scale=-0.5)
                r2 = wkp.tile([128, 1], F32, tag="r2", bufs=2, name="r2")
                nc.vector.tensor_tensor(r2[0:nt], r0[0:nt], r0[0:nt], ALU.mult)
                nc.vector.tensor_tensor(r2[0:nt], r2[0:nt], vv[0:nt], ALU.mult)
                nc.vector.tensor_scalar(r2[0:nt], r2[0:nt], -0.5, 1.5,
                                        ALU.mult, ALU.add)
                rstd = wkp.tile([128, 1], F32, tag="rstd", bufs=2, name="rstd")
                nc.vector.tensor_tensor(rstd[0:nt], r0[0:nt], r2[0:nt], ALU.mult)
                yn = wkp.tile([128, D], BF16, tag="yn", bufs=2, name="yn")
                for nh in range(2):
                    nsl = slice(512 * nh, 512 * nh + 512)
                    t = wkp.tile([128, 512], BF16, tag="lt", bufs=2,
                                 name="lt")
                    nc.vector.tensor_scalar(t[0:nt], py[nh][:],
                                            bn2[0:nt, 0:1], rstd[0:nt],
                                            ALU.subtract, ALU.mult)
                    t2 = wkp.tile([128, 512], BF16, tag="lt2", bufs=2,
                                  name="lt2")
                    nc.vector.tensor_tensor(t2[0:nt], t[0:nt], gbc[0:nt, nsl],
                                            ALU.mult)
                    nc.vector.tensor_tensor(yn[0:nt, nsl], t2[0:nt],
                                            bbc[0:nt, nsl], ALU.add)
                if half is None:
                    nc.sync.dma_start(y_d.ap()[JC * k : JC * k + JC, :],
                                      yn[:])
                else:
                    # half h covers 32-token slices of both batch halves
                    ydst = y_d.ap()[JC * k : JC * k + JC, :].rearrange(
                        "(b t) n -> b t n", b=2)
                    nc.sync.dma_start(
                        ydst[:, 32 * half : 32 * half + 32, :],
                        yn[0:nt, :].rearrange("(b t) n -> b t n", b=2))

            # ---------------- main loop -------------------------------------
            def mkf(fn, *args):
                return lambda: fn(*args)

            pend = []  # deferred divide-epilogues: (it, pc, pxv, rD)

            def drain_divides():
                out = [mkf(emit_divide, *args) for args in pend]
                pend.clear()
                return out

            def emit_stage_a2a(it, half=None):
                ki = it if half is None else 3 + half
                tw = 64 if half is None else 32
                off = 0 if not half else 32
                a2a_dst = a2a_in[ki].rearrange("(j c p) t -> p c j t",
                                               p=128, c=2, j=8)
                src_ap = xvT[:, :, IT * it : IT * it + IT].rearrange(
                    "p c (j t) -> p c j t", j=8)
                for pc in range(2):
                    nc.sync.dma_start(a2a_dst[:, pc],
                                      src_ap[:, pc, :, off : off + tw])
                if sim:
                    # timing stand-in for TimelineSim (no collective support)
                    nc.sync.dma_start(a2a_out[ki][:], a2a_in[ki][:])
                else:
                    nc.gpsimd.collective_compute(
                        "AllToAll", ALU.bypass,
                        replica_groups=groups,
                        ins=[a2a_in[ki].opt()], outs=[a2a_out[ki].opt()])

            for it in range(N_IT):
                q0, q1 = q_next
                nq = [None, None]

                def grab0(itn):
                    nq[0] = emit_q_unit(0, itn)

                def grab1(itn):
                    nq[1] = emit_q_unit(1, itn)

                if it == 0:
                    # prime everything else under the it0 windows; pc0 only
                    # needs v(0,*) (vnat subtiles for heads 0-1), so v(1,*)
                    # rides the pc1 window
                    f0 = {
                        1: [mkf(emit_k_unit, 0, 1)],
                        3: [mkf(emit_v_unit, 0, 1)],
                        5: [mkf(emit_k_unit, 0, 2)],
                        7: [mkf(emit_v_unit, 0, 2)],
                        9: [mkf(emit_k_unit, 0, 3)],
                        11: [mkf(emit_v_unit, 0, 3)],
                        13: [mkf(emit_k_unit, 1, 0)],
                        14: [mkf(emit_v_unit, 1, 0)],
                    }
                    pxv, rD = emit_attention(0, 0, q0, f0)
                    pend.append((0, 0, pxv, rD))
                    q1 = emit_q_unit(1, 0)
                    f1 = {1: [mkf(emit_k_unit, 1, 1)],
                          4: drain_divides(),
                          3: [mkf(emit_v_unit, 1, 1)],
                          5: [mkf(emit_k_unit, 1, 2)],
                          6: [emit_gb_bcast],
                          7: [mkf(emit_v_unit, 1, 2)],
                          8: [mkf(grab0, 1)],
                          9: [mkf(emit_k_unit, 1, 3)],
                          11: [mkf(emit_v_unit, 1, 3)],
                          12: [mkf(grab1, 1)]}
                    pxv, rD = emit_attention(0, 1, q1, f1)
                    pend.append((0, 1, pxv, rD))
                else:
                    # drain prev divide, then exchange + out-project the
                    # previous token block spread over this iteration
                    f0 = {4: drain_divides()
                          + [mkf(emit_stage_a2a, it - 1)]}
                    pxv, rD = emit_attention(it, 0, q0, f0)
                    pend.append((it, 0, pxv, rD))
                    st = emit_outproj_stages(it - 1)
                    f1 = {4: drain_divides() + [st[0]],
                          6: [st[1]], 9: [st[2]], 12: [st[3]]}
                    if it < N_IT - 1:
                        f1[8] = [mkf(grab0, it + 1)]
                        f1[13] = [mkf(grab1, it + 1)]
                    pxv, rD = emit_attention(it, 1, q1, f1)
                    pend.append((it, 1, pxv, rD))
                q_next = nq
            for f in drain_divides():
                f()
            emit_stage_a2a(N_IT - 1, half=0)
            emit_stage_a2a(N_IT - 1, half=1)
            sa = emit_outproj_stages(N_IT - 1, half=0)
            sb = emit_outproj_stages(N_IT - 1, half=1)
            for s in (sa[0], sb[0], sa[1], sa[2], sb[1], sb[2], sa[3], sb[3]):
                s()

    nc.compile()
    return nc


_NC_CACHE = None


def _get_nc():
    global _NC_CACHE
    if _NC_CACHE is None:
        _NC_CACHE = _build()
    return _NC_CACHE


def _prepare_in_maps(x, w_qkv, b_qkv, w_out, b_out, ln_gamma, ln_beta):
    x = np.asarray(x, dtype=np.float32)
    w_qkv = np.asarray(w_qkv, dtype=np.float32)
    b_qkv = np.asarray(b_qkv, dtype=np.float32)
    w_out = np.ascontiguousarray(np.asarray(w_out, dtype=np.float32))
    b_out = np.asarray(b_out, dtype=np.float32)
    ln_gamma = np.asarray(ln_gamma, dtype=np.float32)
    ln_beta = np.asarray(ln_beta, dtype=np.float32)

    cosM, sinM, SpermT = _rope_consts()
    ident = np.eye(128, dtype=np.float32)
    xT = [np.ascontiguousarray(x[b].T) for b in range(B)]

    in_maps = []
    for c in range(NCORES):
        b, g = c // 4, c % 4
        col = slice(256 * g, 256 * g + 256)
        bq = b_qkv[col]
        bk = b_qkv[D:][col]
        bv = b_qkv[2 * D:][col]
        bqkv6 = np.ascontiguousarray(
            np.stack([bq[:128], bq[128:], bk[:128], bk[128:],
                      bv[:128], bv[128:]], axis=1).astype(np.float32))
        # my output tokens: per chunk k, 64 tokens of each batch at
        # 512k + 64c (c = global core id = receiver rank)
        xres = np.concatenate(
            [xT[bb][:, IT * k + 64 * c : IT * k + 64 * c + 64]
             for k in range(N_IT) for bb in range(B)], axis=1)
        m = {
            "xT": xT[b].astype(BF),
            "wq": np.ascontiguousarray(w_qkv[:, col]).astype(BF),
            "wk": np.ascontiguousarray(w_qkv[:, D:][:, col]).astype(BF),
            "wv": np.ascontiguousarray(w_qkv[:, 2 * D:][:, col]).astype(BF),
            "bqkv": bqkv6,
            "wout": w_out.astype(BF),
            "bout": b_out[None, :],
            "gamma": ln_gamma[None, :],
            "beta": ln_beta[None, :],
            "cosM": cosM.astype(BF), "sinM": sinM.astype(BF),
            "SpermT": SpermT, "ident": ident,
            "xres": np.ascontiguousarray(xres).astype(BF),
        }
        in_maps.append(m)
    return in_maps


def _assemble(results):
    out = np.zeros((B, S, D), dtype=np.float32)
    for c in range(NCORES):
        y = results[c]["y_out"]
        for k in range(N_IT):
            for bb in range(B):
                t0 = IT * k + 64 * c
                out[bb, t0 : t0 + 64, :] = \
                    y[JC * k + 64 * bb : JC * k + 64 * bb + 64]
    return out


def run(trace=False, **inputs):
    """Full run returning (output, BassKernelResults) — used by test.py for
    profiling; kernel() below is the graded entry point."""
    in_maps = _prepare_in_maps(**inputs)
    res = bass_utils.run_bass_kernel_spmd(
        _get_nc(), in_maps, core_ids=list(range(NCORES)), trace=trace)
    return _assemble(res.results), res


def kernel(**inputs):
    out, _ = run(trace=False, **inputs)
    return out


# revision 46
# speedup vs baseline: 1.1596x; 1.0063x over previous
"""Multi-head rotary attention block on 8 Trainium2 NeuronCores.

Sharding (data-parallel over batch x tensor-parallel over heads):
  core c: batch b = c//4, head group g = c%4 -> heads 4g..4g+3.
  Each core loads only its batch's x, projects q/k/v for its 4 heads,
  runs attention locally, then a 4-way AllToAll inside each batch quad
  redistributes the attention output from head-sharded to token-sharded form
  for the output projection + layernorm.

The AllToAll is chunked: after each 512-token attention tile completes, one
[1024, 128]-per-core exchange fires and that 128-token slice's output
projection + layernorm runs overlapped with the next attention tile, so the
collective+projection tail is almost fully hidden.

Matmuls run as float32r (full-rate fp32 PE mode, self-loading weights - the
bf16 path would split every matmul into LDWEIGHTS+MATMUL pairs and saturate
the PE sequencer). Softmax is exp(s/32) on ScalarE with denominators
accumulated through an extra ones-column in v, divided out via a fast DVE
reciprocal + PE broadcast. Rope is applied as raw*cos + (SpermT^T raw)*sin
with the rotation permutation as a single matmul per tile; qkv biases ride
the PSUM->SBUF copies as tensor_scalar adds instead of extra matmuls.
"""
import numpy as np
import ml_dtypes

import concourse.bass as bass
import concourse.bacc as bacc
import concourse.tile as tile
import concourse.mybir as mybir
from concourse import bass_utils

F32 = mybir.dt.float32
F32R = mybir.dt.float32r
BF16 = mybir.dt.bfloat16
AF = mybir.ActivationFunctionType
ALU = mybir.AluOpType
BF = ml_dtypes.bfloat16

NCORES = 8
B, S, D = 2, 2048, 1024
HEADS, HD = 16, 64
SCALE = 1.0 / float(np.sqrt(D))  # reference scales by full D, not head_dim
IT = 512          # i-tile width for attention / token block
N_IT = S // IT    # 4
JC = 128          # j-chunk
N_JC = S // JC    # 16
N_EC = D // 128   # 8 e-chunks


def _rope_consts():
    rot = HD // 2
    inv_freq = 1.0 / (10000.0 ** (np.arange(0, rot, 2, dtype=np.float64) / rot))
    ang = np.arange(S, dtype=np.float64)[:, None] * inv_freq[None, :]
    ang = np.repeat(ang, 2, axis=-1)  # [S, 32]
    cos, sin = np.cos(ang), np.sin(ang)
    cosM = np.ones((128, S), dtype=np.float32)
    sinM = np.zeros((128, S), dtype=np.float32)
    for base in (0, 64):
        cosM[base : base + 32, :] = cos.T.astype(np.float32)
        sinM[base : base + 32, :] = sin.T.astype(np.float32)
    Sp = np.zeros((128, 128), dtype=np.float32)
    for base in (0, 64):
        for m in range(32):
            r0 = base + m
            if m % 2 == 0:
                Sp[r0, r0 + 1] = -1.0
            else:
                Sp[r0, r0 - 1] = 1.0
    SpermT = np.ascontiguousarray(Sp.T)
    return cosM, sinM, SpermT


def _build(sim=False):
    nc = bacc.Bacc("TRN2", target_bir_lowering=False, debug=False,
                   num_devices=NCORES)

    xT_d = nc.dram_tensor("xT", [D, S], BF16, kind="ExternalInput")
    wq_d = nc.dram_tensor("wq", [D, 256], BF16, kind="ExternalInput")
    wk_d = nc.dram_tensor("wk", [D, 256], BF16, kind="ExternalInput")
    wv_d = nc.dram_tensor("wv", [D, 256], BF16, kind="ExternalInput")
    bqkv_d = nc.dram_tensor("bqkv", [128, 6], F32, kind="ExternalInput")
    wout_d = nc.dram_tensor("wout", [D, D], BF16, kind="ExternalInput")
    bout_d = nc.dram_tensor("bout", [1, D], F32R, kind="ExternalInput")
    gamma_d = nc.dram_tensor("gamma", [1, D], F32R, kind="ExternalInput")
    beta_d = nc.dram_tensor("beta", [1, D], F32R, kind="ExternalInput")
    cosM_d = nc.dram_tensor("cosM", [128, S], BF16, kind="ExternalInput")
    sinM_d = nc.dram_tensor("sinM", [128, S], BF16, kind="ExternalInput")
    spt_d = nc.dram_tensor("SpermT", [128, 128], F32R, kind="ExternalInput")
    ident_d = nc.dram_tensor("ident", [128, 128], F32R, kind="ExternalInput")
    xres_d = nc.dram_tensor("xres", [D, 512], BF16, kind="ExternalInput")
    y_d = nc.dram_tensor("y_out", [512, D], BF16, kind="ExternalOutput")

    groups = [list(range(NCORES))]

    with tile.TileContext(nc) as tc:
        with (
            tc.tile_pool(name="persist", bufs=1) as pp,
            tc.tile_pool(name="dram", bufs=1, space="DRAM") as dram,
            tc.tile_pool(name="ps", bufs=4, space="PSUM") as ps,
            tc.tile_pool(name="psacc", bufs=2, space="PSUM") as psacc,
            tc.tile_pool(name="wk", bufs=1) as wkp,
        ):
            # ---------------- input DMAs (priority ~ emission order) -------
            wq = pp.tile([128, N_EC, 256], BF16, name="wq_sb")
            wk = pp.tile([128, N_EC, 256], BF16, name="wk_sb")
            wv = pp.tile([128, N_EC, 256], BF16, name="wv_sb")
            cosM = pp.tile([128, S], BF16, name="cosM_sb")
            sinM = pp.tile([128, S], BF16, name="sinM_sb")
            spt = pp.tile([128, 128], F32R, name="spt_sb")
            ident = pp.tile([128, 128], F32R, name="ident_sb")
            bqkv = pp.tile([128, 6], F32, name="bqkv_sb")
            wup_src = pp.tile([1, 512], F32R, name="wup_src")
            nc.vector.memset(wup_src[:].bitcast(F32), 0.125)
            xt = pp.tile([128, N_EC, S], BF16, name="xt_sb")
            xt_src = xT_d.ap().rearrange("(c p) s -> p c s", p=128)

            def w_src(w_dd):
                return w_dd.ap().rearrange("(c p) m -> p c m", p=128)

            # first projection unit (k, pc0, it0) gated only by these:
            nc.sync.dma_start(wk[:], w_src(wk_d))
            nc.sync.dma_start(spt[:], spt_d.ap())
            nc.sync.dma_start(bqkv[:], bqkv_d.ap())
            nc.sync.dma_start(cosM[:], cosM_d.ap())
            nc.sync.dma_start(sinM[:], sinM_d.ap())
            nc.sync.dma_start(xt[:, 0:4, 0:IT], xt_src[:, 0:4, 0:IT])
            nc.sync.dma_start(xt[:, 4:8, 0:IT], xt_src[:, 4:8, 0:IT])
            nc.sync.dma_start(wq[:], w_src(wq_d))
            nc.sync.dma_start(wv[:], w_src(wv_d))
            nc.sync.dma_start(ident[:], ident_d.ap())
            for it in range(1, N_IT):
                isl = slice(IT * it, IT * it + IT)
                nc.sync.dma_start(xt[:, :, isl], xt_src[:, :, isl])

            ones_bf = pp.tile([1, 128], F32R, name="ones_bf")
            nc.vector.memset(ones_bf[:].bitcast(F32), 1.0)
            ones_fr = pp.tile([1, 64], F32R, name="ones_fr")
            nc.vector.memset(ones_fr[:].bitcast(F32), 1.0)
            # PE warmup: dep-free matmuls fill the DMA-bound prolog so the
            # tensor engine reaches full clock before real work arrives
            wup = ps.tile([128, 512], F32, tag="sc", name="wup")
            for _ in range(18):
                nc.tensor.matmul(wup[:], ones_bf[:], wup_src[:],
                                 start=True, stop=True)
            eps_sb = pp.tile([128, 1], F32, name="eps_sb")
            nc.vector.memset(eps_sb[:], 1e-5)

            kT = pp.tile([128, 2, S], F32R, name="kT_sb")
            vnat = [pp.tile([128, 260], F32R, name=f"vnat_{j}")
                    for j in range(N_JC)]
            for j in range(N_JC):
                nc.vector.memset(vnat[j][:, 64::65].bitcast(F32), 1.0)
            xvT = pp.tile([128, 2, S], BF16, name="xvT_sb")

            # ---------------- projection helpers ---------------------------
            # Units are split into phase1 (PSUM accumulation + bias copy) and
            # phase2 (rope / transposes). One phase2 stays pending so the next
            # unit's matmuls fill the PE while DVE finishes the previous
            # unit's bias add - the in-order PE queue never waits on DVE.
            unit_pipe = []

            def pump_units(f2=None):
                while unit_pipe:
                    unit_pipe.pop(0)()
                if f2 is not None:
                    unit_pipe.append(f2)

            def emit_qk_unit(dst_ap, w_sb, bcol, pc, it):
                # dst_ap: [128, 512] destination (bf16 kT slice or f32r q tile)
                isl = slice(IT * it, IT * it + IT)
                praw = ps.tile([128, IT], F32, tag="sc", name="praw")
                for e in range(N_EC):
                    nc.tensor.matmul(praw[:],
                                     w_sb[:, e, 128 * pc : 128 * pc + 128],
                                     xt[:, e, isl],
                                     start=(e == 0), stop=(e == N_EC - 1))
                raw = wkp.tile([128, IT], F32R, tag="raw", bufs=3, name="raw")
                nc.vector.tensor_scalar(raw[:], praw[:],
                                        bqkv[:, bcol : bcol + 1], None, ALU.add)

                def phase2(dst_ap=dst_ap):
                    prot = ps.tile([128, IT], F32, tag="sc", name="prot")
                    nc.tensor.matmul(prot[:], spt[:], raw[:],
                                     start=True, stop=True)
                    t1 = wkp.tile([128, IT], BF16, tag="t1", bufs=2,
                                  name="t1")
                    nc.vector.tensor_tensor(t1[:], prot[:], sinM[:, isl],
                                            ALU.mult)
                    t2 = wkp.tile([128, IT], BF16, tag="t2", bufs=2,
                                  name="t2")
                    nc.gpsimd.tensor_tensor(t2[:], raw[:].bitcast(F32),
                                            cosM[:, isl], ALU.mult)
                    nc.vector.tensor_tensor(dst_ap, t1[:], t2[:],
                                            ALU.add)

                pump_units(phase2)

            def emit_q_unit(pc, it):
                q_t = wkp.tile([128, IT], F32R, tag="qt", bufs=3, name="q_t")
                emit_qk_unit(q_t[:], wq, pc, pc, it)
                return q_t

            def emit_v_unit(pc, it):
                isl = slice(IT * it, IT * it + IT)
                pvt = ps.tile([128, IT], F32, tag="sc", name="pvt")
                for e in range(N_EC):
                    nc.tensor.matmul(pvt[:],
                                     wv[:, e, 128 * pc : 128 * pc + 128],
                                     xt[:, e, isl],
                                     start=(e == 0), stop=(e == N_EC - 1))
                vt = wkp.tile([128, IT], F32R, tag="vt", bufs=2, name="vt")
                nc.vector.tensor_scalar(vt[:], pvt[:],
                                        bqkv[:, 4 + pc : 5 + pc], None, ALU.add)

                def phase2():
                    for jj in range(IT // JC):
                        jcc = it * (IT // JC) + jj
                        ptr = ps.tile([128, 128], F32R, tag="sc", name="ptr")
                        nc.tensor.transpose(
                            ptr[:], vt[:, JC * jj : JC * jj + JC], ident[:])
                        for hh in range(2):
                            h = 2 * pc + hh
                            nc.vector.tensor_copy(
                                vnat[jcc][:, 65 * h : 65 * h + 64],
                                ptr[:, 64 * hh : 64 * hh + 64].bitcast(F32))

                pump_units(phase2)

            # ---------------- attention helper ------------------------------
            def emit_attention(it, pc, q_t, fillers=None):
                pump_units()
                isl = slice(IT * it, IT * it + IT)
                pxv = psacc.tile([128, 1024], F32, tag="acc", name="pxv")
                for jc in range(N_JC):
                    pump_units()  # pending phase2 lands 1 j-chunk after its
                    if fillers and jc in fillers:  # phase1 - always in time
                        for f in fillers[jc]:
                            f()
                    jsl = slice(JC * jc, JC * jc + JC)
                    for hh in range(2):
                        h = 2 * pc + hh
                        hsl = slice(64 * hh, 64 * hh + 64)
                        psc = ps.tile([128, IT], F32, tag="sc", name="psc")
                        nc.tensor.matmul(psc[:], kT[hsl, pc, jsl],
                                         q_t[hsl, :],
                                         start=True, stop=True)
                        pt = wkp.tile([128, IT], F32R, tag="pt", bufs=6,
                                      name="pt")
                        nc.scalar.activation(pt[:], psc[:], AF.Exp, scale=SCALE)
                        nc.tensor.matmul(
                            pxv[0:65, 512 * hh : 512 * hh + 512],
                            vnat[jc][:, 65 * h : 65 * h + 65],
                            pt[:], start=(jc == 0), stop=(jc == N_JC - 1))
                # softmax denominator reciprocal (broadcast + multiply are
                # deferred into the next tile-group's fillers so their
                # dep-stalls never block the in-order PE queue)
                rDf = wkp.tile([1, 1024], F32, tag="rdf", bufs=1, name="rDf")
                nc.vector.reciprocal_approx_fast(rDf[:], pxv[64:65, :])
                rD = wkp.tile([1, 1024], F32R, tag="rd", bufs=1, name="rD")
                nc.gpsimd.tensor_copy(rD[:], rDf[:])
                return pxv, rD

            def emit_divide(it, pc, pxv, rD):
                isl = slice(IT * it, IT * it + IT)
                for hh in range(2):
                    nsl = slice(512 * hh, 512 * hh + 512)
                    rDb = ps.tile([128, IT], F32, tag="sc", name="rDb")
                    nc.tensor.matmul(rDb[0:64, :], ones_fr[:], rD[:, nsl],
                                     start=True, stop=True)
                    rDs = wkp.tile([64, IT], BF16, tag="rds", bufs=4,
                                   name="rDs")
                    nc.vector.tensor_copy(rDs[:], rDb[0:64, :])
                    nc.vector.tensor_tensor(
                        xvT[64 * hh : 64 * hh + 64, pc, isl],
                        pxv[0:64, nsl], rDs[:], ALU.mult)

            # ---------------- emit: first units, rest via fillers -----------
            def emit_k_unit(pc, it):
                emit_qk_unit(kT[:, pc, IT * it : IT * it + IT],
                             wk, 2 + pc, pc, it)

            emit_k_unit(0, 0)
            emit_v_unit(0, 0)
            q_next = [emit_q_unit(0, 0), None]

            # late-phase inputs: DMAs emitted early (low queue priority is
            # fine - only out-projection needs them), broadcast matmuls
            # deferred into an it0 filler so they never stall the PE queue.
            wout = pp.tile([128, N_EC, D], BF16, name="wout_sb")
            bout = pp.tile([1, D], F32R, name="bout_sb")
            gamma = pp.tile([1, D], F32R, name="gamma_sb")
            beta = pp.tile([1, D], F32R, name="beta_sb")
            xres = pp.tile([128, N_EC, 512], BF16, name="xres_sb")
            gbc = pp.tile([128, D], BF16, name="gbc_sb")
            bbc = pp.tile([128, D], BF16, name="bbc_sb")
            nc.sync.dma_start(
                wout[:], wout_d.ap().rearrange("(c p) n -> p c n", p=128))
            nc.sync.dma_start(bout[:], bout_d.ap())
            nc.sync.dma_start(gamma[:], gamma_d.ap())
            nc.sync.dma_start(beta[:], beta_d.ap())
            nc.sync.dma_start(
                xres[:], xres_d.ap().rearrange("(c p) s -> p c s", p=128))

            def emit_gb_bcast():
                for src_t, dst in ((gamma, gbc), (beta, bbc)):
                    for half in range(2):
                        nsl = slice(512 * half, 512 * half + 512)
                        pbc = ps.tile([128, 512], F32, tag="sc", name="pbc")
                        nc.tensor.matmul(pbc[:], ones_bf[:], src_t[:, nsl],
                                         start=True, stop=True)
                        nc.vector.tensor_copy(dst[:, nsl], pbc[:])

            # 8-way exchange: sender block j = [256 chan, 64 tok] slice j
            # of its batch; receiver j gets batch-0 channels (senders 0-3)
            # in rows 0-1023 and batch-1 (senders 4-7) in rows 1024-2047.
            a2a_in = [dram.tile([2048, 64], BF16, name=f"a2a_in{k}")
                      for k in range(N_IT)]
            a2a_out = [dram.tile([2048, 64], BF16, name=f"a2a_out{k}")
                       for k in range(N_IT)]

            # ---------------- out-projection + layernorm chunk --------------
            def emit_outproj_stages(k, half=None):
                """Out-projection for token block k as a list of (slot, fn)
                emissions so the zk DMA latency and the matmul burst spread
                over several j-chunks instead of stalling the PE queue."""
                ki = k if half is None else 3 + half
                tw = 64 if half is None else 32
                nt = 2 * tw
                zk = wkp.tile([128, N_EC, nt], BF16, tag="zk", bufs=2,
                              name="zk")
                py = [ps.tile([nt, 512], F32, tag="sc", name=f"py{nh}")
                      for nh in range(2)]

                def s_load():
                    zsrc = a2a_out[ki].rearrange("(b e p) t -> p b e t",
                                                 p=128, b=2, e=N_EC)
                    for bh in range(2):
                        tsl = slice(tw * bh, tw * bh + tw)
                        xoff = JC * k + 64 * bh + (32 * half if half else 0)
                        nc.sync.dma_start(zk[:, :, tsl], zsrc[:, bh])
                        nc.gpsimd.tensor_tensor(
                            zk[:, :, tsl], zk[:, :, tsl],
                            xres[:, :, xoff : xoff + tw], ALU.add)

                def s_mm(nh):
                    nsl = slice(512 * nh, 512 * nh + 512)
                    for e in range(N_EC):
                        nc.tensor.matmul(py[nh][:], zk[:, e, :],
                                         wout[:, e, nsl],
                                         start=(e == 0), stop=False)
                    nc.tensor.matmul(py[nh][:], ones_bf[:, 0:nt],
                                     bout[:, nsl], start=False, stop=True)

                def s_ln():
                    emit_ln_store(k, py, half)

                return [s_load, lambda: s_mm(0), lambda: s_mm(1), s_ln]

            def emit_ln_store(k, py, half=None):
                tw = 64 if half is None else 32
                nt = 2 * tw
                bn6 = wkp.tile([128, 2, 6], F32, tag="bn6", bufs=2, name="bn6")
                nc.vector.bn_stats(bn6[0:nt, 0, :], py[0][:])
                nc.vector.bn_stats(bn6[0:nt, 1, :], py[1][:])
                bn2 = wkp.tile([128, 2], F32, tag="bn2", bufs=2, name="bn2")
                nc.vector.bn_aggr(bn2[0:nt], bn6[0:nt])
                # rstd = (var+eps)^-0.5 without Ln (keeps ScalarE on the Exp
                # table the whole kernel): Mitchell bitwise log2 on DVE ->
                # exp(-0.5 ln v) seed -> one Newton step to 3e-4 accuracy.
                vv = wkp.tile([128, 1], F32, tag="lnv", bufs=2, name="vv")
                nc.vector.tensor_scalar(vv[0:nt], bn2[0:nt, 1:2], 1e-5, None,
                                        ALU.add)
                iv = wkp.tile([128, 1], F32, tag="iv", bufs=2, name="iv")
                nc.vector.tensor_copy(iv[0:nt], vv[0:nt].bitcast(mybir.dt.int32))
                lnv = wkp.tile([128, 1], F32, tag="lnv2", bufs=2, name="lnv")
                LN2 = float(np.log(2.0))
                nc.vector.tensor_scalar(lnv[0:nt], iv[0:nt], LN2 / (1 << 23),
                                        -(127.0 - 0.0450) * LN2,
                                        ALU.mult, ALU.add)
                r0 = wkp.tile([128, 1], F32, tag="rstd0", bufs=2, name="r0")
                nc.scalar.activation(r0[0:nt], lnv[0:nt], AF.Exp, scale=-0.5)
                r2 = wkp.tile([128, 1], F32, tag="r2", bufs=2, name="r2")
                nc.vector.tensor_tensor(r2[0:nt], r0[0:nt], r0[0:nt], ALU.mult)
                nc.vector.tensor_tensor(r2[0:nt], r2[0:nt], vv[0:nt], ALU.mult)
                nc.vector.tensor_scalar(r2[0:nt], r2[0:nt], -0.5, 1.5,
                                        ALU.mult, ALU.add)
                rstd = wkp.tile([128, 1], F32, tag="rstd", bufs=2, name="rstd")
                nc.vector.tensor_tensor(rstd[0:nt], r0[0:nt], r2[0:nt], ALU.mult)
                yn = wkp.tile([128, D], BF16, tag="yn", bufs=2, name="yn")
                for nh in range(2):
                    nsl = slice(512 * nh, 512 * nh + 512)
                    t = wkp.tile([128, 512], BF16, tag="lt", bufs=2,
                                 name="lt")
                    nc.vector.tensor_scalar(t[0:nt], py[nh][:],
                                            bn2[0:nt, 0:1], rstd[0:nt],
                                            ALU.subtract, ALU.mult)
                    t2 = wkp.tile([128, 512], BF16, tag="lt2", bufs=2,
                                  name="lt2")
                    nc.vector.tensor_tensor(t2[0:nt], t[0:nt], gbc[0:nt, nsl],
                                            ALU.mult)
                    nc.vector.tensor_tensor(yn[0:nt, nsl], t2[0:nt],
                                            bbc[0:nt, nsl], ALU.add)
                if half is None:
                    nc.sync.dma_start(y_d.ap()[JC * k : JC * k + JC, :],
                                      yn[:])
                else:
                    # half h covers 32-token slices of both batch halves
                    ydst = y_d.ap()[JC * k : JC * k + JC, :].rearrange(
                        "(b t) n -> b t n", b=2)
                    nc.sync.dma_start(
                        ydst[:, 32 * half : 32 * half + 32, :],
                        yn[0:nt, :].rearrange("(b t) n -> b t n", b=2))

            # ---------------- main loop -------------------------------------
            def mkf(fn, *args):
                return lambda: fn(*args)

            pend = []  # deferred divide-epilogues: (it, pc, pxv, rD)

            def drain_divides():
                out = [mkf(emit_divide, *args) for args in pend]
                pend.clear()
                return out

            def emit_stage_a2a(it, half=None):
                ki = it if half is None else 3 + half
                tw = 64 if half is None else 32
                off = 0 if not half else 32
                a2a_dst = a2a_in[ki].rearrange("(j c p) t -> p c j t",
                                               p=128, c=2, j=8)
                src_ap = xvT[:, :, IT * it : IT * it + IT].rearrange(
                    "p c (j t) -> p c j t", j=8)
                for pc in range(2):
                    nc.sync.dma_start(a2a_dst[:, pc],
                                      src_ap[:, pc, :, off : off + tw])
                if sim:
                    # timing stand-in for TimelineSim (no collective support)
                    nc.sync.dma_start(a2a_out[ki][:], a2a_in[ki][:])
                else:
                    nc.gpsimd.collective_compute(
                        "AllToAll", ALU.bypass,
                        replica_groups=groups,
                        ins=[a2a_in[ki].opt()], outs=[a2a_out[ki].opt()])

            for it in range(N_IT):
                q0, q1 = q_next
                nq = [None, None]

                def grab0(itn):
                    nq[0] = emit_q_unit(0, itn)

                def grab1(itn):
                    nq[1] = emit_q_unit(1, itn)

                if it == 0:
                    # prime everything else under the it0 windows; pc0 only
                    # needs v(0,*) (vnat subtiles for heads 0-1), so v(1,*)
                    # rides the pc1 window
                    f0 = {
                        1: [mkf(emit_k_unit, 0, 1)],
                        3: [mkf(emit_v_unit, 0, 1)],
                        5: [mkf(emit_k_unit, 0, 2)],
                        7: [mkf(emit_v_unit, 0, 2)],
                        9: [mkf(emit_k_unit, 0, 3)],
                        11: [mkf(emit_v_unit, 0, 3)],
                        13: [mkf(emit_k_unit, 1, 0)],
                        14: [mkf(emit_v_unit, 1, 0)],
                    }
                    pxv, rD = emit_attention(0, 0, q0, f0)
                    pend.append((0, 0, pxv, rD))
                    q1 = emit_q_unit(1, 0)
                    f1 = {1: [mkf(emit_k_unit, 1, 1)],
                          4: drain_divides(),
                          3: [mkf(emit_v_unit, 1, 1)],
                          5: [mkf(emit_k_unit, 1, 2)],
                          6: [emit_gb_bcast],
                          7: [mkf(emit_v_unit, 1, 2)],
                          8: [mkf(grab0, 1)],
                          9: [mkf(emit_k_unit, 1, 3)],
                          11: [mkf(emit_v_unit, 1, 3)],
                          12: [mkf(grab1, 1)]}
                    pxv, rD = emit_attention(0, 1, q1, f1)
                    pend.append((0, 1, pxv, rD))
                else:
                    # drain prev divide, then exchange + out-project the
                    # previous token block spread over this iteration
                    f0 = {4: drain_divides()
                          + [mkf(emit_stage_a2a, it - 1)]}
                    pxv, rD = emit_attention(it, 0, q0, f0)
                    pend.append((it, 0, pxv, rD))
                    st = emit_outproj_stages(it - 1)
                    f1 = {4: drain_divides() + [st[0]],
                          6: [st[1]], 9: [st[2]], 12: [st[3]]}
                    if it < N_IT - 1:
                        f1[8] = [mkf(grab0, it + 1)]
                        f1[13] = [mkf(grab1, it + 1)]
                    pxv, rD = emit_attention(it, 1, q1, f1)
                    pend.append((it, 1, pxv, rD))
                q_next = nq
            for f in drain_divides():
                f()
            emit_stage_a2a(N_IT - 1, half=0)
            emit_stage_a2a(N_IT - 1, half=1)
            sa = emit_outproj_stages(N_IT - 1, half=0)
            sb = emit_outproj_stages(N_IT - 1, half=1)
            for s in (sa[0], sb[0], sa[1], sa[2], sb[1], sb[2], sa[3], sb[3]):
                s()

    nc.compile()
    return nc


_NC_CACHE = None


def _get_nc():
    global _NC_CACHE
    if _NC_CACHE is None:
        _NC_CACHE = _build()
    return _NC_CACHE


def _prepare_in_maps(x, w_qkv, b_qkv, w_out, b_out, ln_gamma, ln_beta):
    x = np.asarray(x, dtype=np.float32)
    w_qkv = np.asarray(w_qkv, dtype=np.float32)
    b_qkv = np.asarray(b_qkv, dtype=np.float32)
    w_out = np.ascontiguousarray(np.asarray(w_out, dtype=np.float32))
    b_out = np.asarray(b_out, dtype=np.float32)
    ln_gamma = np.asarray(ln_gamma, dtype=np.float32)
    ln_beta = np.asarray(ln_beta, dtype=np.float32)

    cosM, sinM, SpermT = _rope_consts()
    ident = np.eye(128, dtype=np.float32)
    xT = [np.ascontiguousarray(x[b].T) for b in range(B)]

    in_maps = []
    for c in range(NCORES):
        b, g = c // 4, c % 4
        col = slice(256 * g, 256 * g + 256)
        bq = b_qkv[col]
        bk = b_qkv[D:][col]
        bv = b_qkv[2 * D:][col]
        bqkv6 = np.ascontiguousarray(
            np.stack([bq[:128], bq[128:], bk[:128], bk[128:],
                      bv[:128], bv[128:]], axis=1).astype(np.float32))
        # my output tokens: per chunk k, 64 tokens of each batch at
        # 512k + 64c (c = global core id = receiver rank)
        xres = np.concatenate(
            [xT[bb][:, IT * k + 64 * c : IT * k + 64 * c + 64]
             for k in range(N_IT) for bb in range(B)], axis=1)
        m = {
            "xT": xT[b].astype(BF),
            "wq": np.ascontiguousarray(w_qkv[:, col]).astype(BF),
            "wk": np.ascontiguousarray(w_qkv[:, D:][:, col]).astype(BF),
            "wv": np.ascontiguousarray(w_qkv[:, 2 * D:][:, col]).astype(BF),
            "bqkv": bqkv6,
            "wout": w_out.astype(BF),
            "bout": b_out[None, :],
            "gamma": ln_gamma[None, :],
            "beta": ln_beta[None, :],
            "cosM": cosM.astype(BF), "sinM": sinM.astype(BF),
            "SpermT": SpermT, "ident": ident,
            "xres": np.ascontiguousarray(xres).astype(BF),
        }
        in_maps.append(m)
    return in_maps


def _assemble(results):
    out = np.zeros((B, S, D), dtype=np.float32)
    for c in range(NCORES):
        y = results[c]["y_out"]
        for k in range(N_IT):
            for bb in range(B):
                t0 = IT * k + 64 * c
                out[bb, t0 : t0 + 64, :] = \
                    y[JC * k + 64 * bb : JC * k + 64 * bb + 64]
    return out


def run(trace=False, **inputs):
    """Full run returning (output, BassKernelResults) — used by test.py for
    profiling; kernel() below is the graded entry point."""
    in_maps = _prepare_in_maps(**inputs)
    res = bass_utils.run_bass_kernel_spmd(
        _get_nc(), in_maps, core_ids=list(range(NCORES)), trace=trace)
    return _assemble(res.results), res


def kernel(**inputs):
    out, _ = run(trace=False, **inputs)
    return out
